# revision 46
# baseline (speedup 1.0000x reference)
"""Trainium2 Bass kernel for the char-LSTM word-similarity CNN scorer.

Problem: B=8192 examples x NW=4 words x L=16 chars. Per word: char
embeddings -> masked LSTMCell over <=16 steps -> cell state c [128].
Per example: 4x4 cosine matrix of the word reps -> 2-layer 2x2-valid
CNN -> linear scorer -> sigmoid.

Strategy (pure data parallel, 1024 examples / 4096 words per core):
 - Host folds emb @ W_ih.T + (b_ih + b_hh) into a [128, 512] table
   (row 64 = "freeze" flag driving f->1, i->0 for words past their
   length); per-step char inputs are a one-hot matrix.
 - Words sorted by length (desc) on host; step t processes exactly the
   active width N_t (rounded to 16), in 512-column PSUM blocks.
 - fp8e4 DoubleRow matmuls: each gate's pre-activation is ONE PE
   matmul with two k-tiles -- (one-hot @ G65) + (h @ WhhT) -- at 0.5
   cycles per output column (plus an optional residual-table pass that
   cancels the fp8 weight quantization error). The one-hot operand is
   exact in fp8; h is stored as fp8e4 in the same SBUF tile as the
   one-hot so both k-tiles come from a single strided access pattern.
 - ACT work minimized: ALL FOUR gate activations run as ONE strided-AP
   sigmoid per block (g-gate table rows are pre-doubled so tanh(g) =
   2*sigmoid(2g)-1; the fixup is a cheap DVE tensor_scalar);
   o-gate/tanh(c)/h only computed for words still alive at the next
   step; no f-gate at t=0 (c0=0); no memsets (c=i*g at t=0, h never
   read at t=0). The sigmoid output buffer is double-buffered by step
   parity so it never write-after-read stalls against DVE.
 - PSUM: 4-bank gate groups (i|f|o|g), double buffered. Dying blocks
   park g in the o-bank so their own g-bank is a dependency-free PE
   transpose target.
 - As each 512-word block freezes, its c columns are transposed
   ([H,word]->[word,H]) and indirect-DMA scattered to DRAM in original
   word order -- overlapped with the remaining LSTM steps. Chunks of
   the last block past W[15] are provably frozen a step early and
   pre-scattered. The tail is then: one contiguous gather, all 10 dot
   products (6 pairs + 4 self-dots) on DVE, rsqrt norm scaling, and
   the tiny CNN/scorer lowered to matmuls pipelined in two
   example-halves.
"""

import os
import sys

for _p in ("/opt/trn_rl_repo",):
    if _p not in sys.path and os.path.isdir(_p):
        sys.path.insert(0, _p)

import numpy as np
import ml_dtypes

import concourse.bass as bass
import concourse.mybir as mybir
import concourse.tile as tile
from concourse.bass_utils import run_bass_kernel_spmd
from concourse.masks import make_identity

F8NP = ml_dtypes.float8_e4m3

# This container's walrus build rejects CTRL instructions (Drain) carrying
# more than 2 sync waits ("Too many sync wait commands" in setupSyncWait).
# Tile's kernel-tail drain accumulates one wait per engine/DMA-queue sem, so
# redistribute: keep one wait on the drain, move the rest onto nofuse NOPs
# that execute before the all-engine barrier. Semantics are unchanged (all
# waits still complete before the barrier / semaphore teardown).
def _patched_drain_and_barrier(self, tick_clock, wait_clock):
    nc = self.nc
    drain_inst = nc.sync.drain()
    wait_clock.add_sem_waits(
        drain_inst.ins, tile.ScopedClock({None: tick_clock.global_clock})
    )
    waits = list(drain_inst.ins.sync_info.on_wait)
    if len(waits) > 1:
        drain_inst.ins.sync_info.on_wait = waits[:1]
        for k in range(1, len(waits)):
            nop = nc.sync.nop(nofuse=True, hint="drain_wait_spill")
            if nop.ins.sync_info is None:
                nop.ins.sync_info = mybir.SyncInfo(on_wait=[], on_update=[])
            nop.ins.sync_info.on_wait = [waits[k]]
    nc.all_engine_barrier()
    assert self.sems is not None
    popped = nc._tile_sem_poison_stack.pop()
    assert popped is self._sem_poison
    nc.clear_and_free_semaphores(list(self.sems.allocated().values()))
    nc.all_engine_barrier()


tile.TileContext._drain_and_barrier = _patched_drain_and_barrier

def _spill_excess_waits(nc):
    """Walrus here rejects instructions with more than ~2 sync waits. Spill
    excess waits onto same-engine NoOps inserted just before the instruction
    (engines dispatch in program order, so waiting earlier on the same engine
    is equivalent)."""
    cnt = [0]
    for fn in nc.m.functions:
        for bb in fn.blocks:
            insts = list(bb.instructions)
            out = []
            changed = False
            for inst in insts:
                si = inst.sync_info
                waits = list(si.on_wait) if si is not None and si.on_wait else []
                max_waits = 1
                if len(waits) > max_waits:
                    changed = True
                    keep = waits[-max_waits:]
                    extra = waits[:-max_waits]
                    for j in range(0, len(extra), max_waits):
                        cnt[0] += 1
                        nop = mybir.InstNoOp(name=f"I-spillw-{cnt[0]}", ins=[], outs=[])
                        nop.engine = inst.engine
                        nop.sync_info = mybir.SyncInfo(
                            on_wait=extra[j:j + max_waits], on_update=[])
                        nop.bass_nofuse = True
                        nop.bass_priority = 0
                        nop.text_hint = "spillw"
                        nop.debug = inst.debug
                        out.append(nop)
                    si.on_wait = keep
                out.append(inst)
            if changed:
                bb.instructions = out

B, NW, L, E, H, V = 8192, 4, 16, 128, 128, 64
NCORES = 8
PER = B // NCORES          # 1024 examples per core
NWORD = PER * NW           # 4096 words per core
NEC = PER // 128           # 8 example-chunks of 128
BLK = 512
NBLK = NWORD // BLK
FB = 30.0                  # freeze bias magnitude
RES = False                # second DoubleRow pass with fp8 residual tables
F32 = mybir.dt.float32
F16 = mybir.dt.float16
F8 = mybir.dt.float8e4
I32 = mybir.dt.int32
AF = mybir.ActivationFunctionType
ALU = mybir.AluOpType
DR = mybir.MatmulPerfMode.DoubleRow

P6 = [(0, 1), (0, 2), (0, 3), (1, 2), (1, 3), (2, 3)]


# ----------------------------------------------------------------- host prep

def _f8rt(x):
    return x.astype(F8NP).astype(np.float32)


def _build_consts(inp):
    emb = np.asarray(inp["emb_i"], np.float32)
    W_ih = np.asarray(inp["W_ih"], np.float32)
    W_hh = np.asarray(inp["W_hh"], np.float32)
    b = np.asarray(inp["b_ih"], np.float32) + np.asarray(inp["b_hh"], np.float32)
    G65 = np.zeros((128, 4 * H), np.float32)
    G65[:V] = emb @ W_ih.T + b
    G65[V, 0:H] = -FB
    G65[V, H:2 * H] = +FB
    WhhT = np.ascontiguousarray(W_hh.T)  # [H, 4H]
    # g-gate pre-activations are doubled so tanh(g) can be evaluated on the
    # sigmoid table together with i,f,o in one strided ACT instruction:
    # tanh(x) = 2*sigmoid(2x) - 1 (the 2s-1 fixup runs on DVE)
    G65[:, 2 * H:3 * H] *= 2.0
    WhhT[:, 2 * H:3 * H] *= 2.0

    # interleaved DoubleRow lhsT tables [128, 2, 4H]: k-tile 0 = G65
    # (one-hot side), k-tile 1 = WhhT (h side); plus fp8 residuals.
    LT = np.zeros((128, 2, 4 * H), np.float32)
    LTR = np.zeros((128, 2, 4 * H), np.float32)
    LT[:, 0, :] = _f8rt(G65)
    LT[:, 1, :] = _f8rt(WhhT)
    LTR[:, 0, :] = _f8rt(G65 - LT[:, 0, :])
    LTR[:, 1, :] = _f8rt(WhhT - LT[:, 1, :])

    w1 = np.asarray(inp["conv1_w"], np.float32)
    b1 = np.asarray(inp["conv1_b"], np.float32)
    w2 = np.asarray(inp["conv2_w"], np.float32)
    b2 = np.asarray(inp["conv2_b"], np.float32)
    ws = np.asarray(inp["scorer_w"], np.float32)
    bs = float(np.asarray(inp["scorer_b"], np.float32)[0])

    p6idx = {p: i for i, p in enumerate(P6)}
    W1eff = np.zeros((6, 36), np.float32)
    b1eff = np.zeros((36, 1), np.float32)
    for c in range(4):
        for y in range(3):
            for x in range(3):
                m = c * 9 + y * 3 + x
                b1eff[m, 0] += b1[c]
                for dy in range(2):
                    for dx in range(2):
                        a, bb = y + dy, x + dx
                        w = w1[c, 0, dy, dx]
                        if a == bb:
                            b1eff[m, 0] += w
                        else:
                            W1eff[p6idx[(min(a, bb), max(a, bb))], m] += w
    W2eff = np.zeros((36, 32), np.float32)
    b2eff = np.zeros((32, 1), np.float32)
    for c2 in range(8):
        for y in range(2):
            for x in range(2):
                m = c2 * 4 + y * 2 + x
                b2eff[m, 0] = b2[c2]
                for c1 in range(4):
                    for dy in range(2):
                        for dx in range(2):
                            W2eff[c1 * 9 + (y + dy) * 3 + (x + dx), m] += w2[c2, c1, dy, dx]
    Wsc = ws[0].astype(np.float32).reshape(32, 1)
    return dict(LT=LT, LTR=LTR, W1eff=W1eff, b1eff=b1eff,
                W2eff=W2eff, b2eff=b2eff, Wsc=Wsc, bsc=bs)


def _core_prep(word_ids_c, lengths_c):
    wid = np.asarray(word_ids_c).reshape(NWORD, L)
    lens = np.asarray(lengths_c).reshape(NWORD)
    perm = np.argsort(-lens, kind="stable").astype(np.int32)
    wid_s = wid[perm]
    lens_s = lens[perm]
    Nt = (np.arange(L)[:, None] < lens_s[None, :]).sum(1)  # [L]
    return wid_s, lens_s, Nt, perm


def _build_onehot(wid_s, lens_s, widths):
    oh = np.zeros((L, 128, NWORD), F8NP)
    one = np.ones((), F8NP)
    cols = np.arange(NWORD)
    for t in range(L):
        n = int(widths[t])
        if n == 0:
            continue
        alive = lens_s[:n] > t
        oh[t, wid_s[:n, t], cols[:n]] = np.where(alive, one, np.zeros((), F8NP))
        oh[t, V, cols[:n]] = np.where(alive, np.zeros((), F8NP), one)
    return oh


def _build_scidx(perm):
    # scidx[p, cc] = original index of sorted word cc*128+p
    return np.ascontiguousarray(
        perm.reshape(NWORD // 128, 128).T).astype(np.int32)


# -------------------------------------------------------------- bass program

def _build_program(W):
    """W: tuple of per-step active widths (len 16, multiples of 16, desc)."""
    nc = bass.Bass()
    Wx = list(W) + [0]

    oh_in = nc.dram_tensor("oh", [L, 128, NWORD], F8, kind="ExternalInput")
    sci_in = nc.dram_tensor("scidx", [128, NWORD // 128], I32, kind="ExternalInput")
    lt_in = nc.dram_tensor("lt", [128, 2, 4 * H], F8, kind="ExternalInput")
    ltr_in = nc.dram_tensor("ltr", [128, 2, 4 * H], F8, kind="ExternalInput")
    w1_in = nc.dram_tensor("w1", [6, 36], F16, kind="ExternalInput")
    b1_in = nc.dram_tensor("b1", [36, 1], F32, kind="ExternalInput")
    w2_in = nc.dram_tensor("w2", [36, 32], F16, kind="ExternalInput")
    b2_in = nc.dram_tensor("b2", [32, 1], F32, kind="ExternalInput")
    wsc_in = nc.dram_tensor("wsc", [32, 1], F16, kind="ExternalInput")
    bsc_in = nc.dram_tensor("bsc", [1, 1], F32, kind="ExternalInput")
    out_d = nc.dram_tensor("out", [1, PER], F32, kind="ExternalOutput")
    c_dram = nc.dram_tensor("cscratch", [NWORD, H], F16)

    def view(ap, off, dims):
        return bass.AP(ap.tensor, ap.offset + off, [ap.ap[0]] + dims)

    with tile.TileContext(nc) as tc:
        with (
            tc.tile_pool(name="const", bufs=1) as cpool,
            tc.tile_pool(name="state", bufs=1) as spool,
        ):
            lt_sb = cpool.tile([128, 2, 4 * H], F8, tag="lt", name="lt")
            ltr_sb = cpool.tile([128, 2, 4 * H], F8, tag="ltr", name="ltr") \
                if RES else None
            w1_sb = cpool.tile([6, 36], F16, tag="w1", name="w1")
            b1_sb = cpool.tile([36, 1], F32, tag="b1", name="b1")
            w2_sb = cpool.tile([36, 32], F16, tag="w2", name="w2")
            b2_sb = cpool.tile([32, 1], F32, tag="b2", name="b2")
            wsc_sb = cpool.tile([32, 1], F16, tag="wsc", name="wsc")
            bsc_sb = cpool.tile([1, 1], F32, tag="bsc", name="bsc")
            ident = cpool.tile([128, 128], F16, tag="ident", name="ident")
            sci_sb = cpool.tile([128, NWORD // 128], I32, tag="sci", name="sci")

            # xh: slot 0 = per-step one-hot (DMA), slot 1 = h state (fp8)
            xh = spool.tile([128, 2, NWORD], F8, tag="xh", name="xh")
            c_sb = spool.tile([128, NWORD], F16, tag="c", name="c")
            # sifo slots: 0=sig(i) 1=sig(f) 2=sig(o) 3=sig(2g), double
            # buffered by step parity so step t+1's sigmoid never waits for
            # DVE's reads of step t's values (write-after-read)
            sifo = spool.tile([128, 8 * NWORD], F16, tag="sifo", name="sifo")
            sg = spool.tile([128, NWORD], F16, tag="sg", name="sg")
            sc = spool.tile([128, NWORD], F16, tag="sc", name="sc")

            # startup: the critical first-matmul inputs (lt + step-0 one-hot)
            # issue first, spread over engine queues that are idle at t=0 so
            # the ~900ns-per-issue DMA costs overlap. Everything else follows
            # on sync/gpsimd.
            nc.sync.dma_start(lt_sb[:], lt_in[:])
            w0 = Wx[0]
            nb0 = (w0 + BLK - 1) // BLK
            eng0 = [nc.scalar, nc.gpsimd, nc.sync, nc.scalar,
                    nc.gpsimd, nc.sync, nc.scalar, nc.gpsimd]
            for k in range(nb0):
                a, b = k * BLK, min(w0, (k + 1) * BLK)
                eng0[k].dma_start(xh[:, 0, a:b], oh_in[0, :, a:b])
            consts = [(w1_sb, w1_in), (b1_sb, b1_in), (w2_sb, w2_in),
                      (b2_sb, b2_in), (wsc_sb, wsc_in), (bsc_sb, bsc_in),
                      (sci_sb, sci_in)]
            if RES:
                consts.append((ltr_sb, ltr_in))
            for sb, dr in consts:
                nc.gpsimd.dma_start(sb[:], dr[:])

            # ------------------------------------------------ LSTM main loop
            with (
                tc.tile_pool(name="g4", bufs=2, space="PSUM") as g4pool,
                tc.tile_pool(name="cn", bufs=4) as cnpool,
            ):
                # gate column offsets inside a [128, 2048] 4-bank group
                CI, CF, CO, CG = 0, 512, 1024, 1536
                GS = {0: slice(0, H), 1: slice(H, 2 * H),
                      2: slice(2 * H, 3 * H), 3: slice(3 * H, 4 * H)}
                oready = []  # [(base, wkn)] tanh(c)+h pending
                # chunk c (cols [128c, 128c+128)) is provably frozen in
                # every core once the next step's width cannot reach it;
                # finalize (transpose+scatter) each chunk at that step so
                # the WAW-serialized scatter chain drains early
                NCH = NWORD // 128
                fstep = [min(t for t in range(L)
                             if Wx[t + 1] <= 128 * c) for c in range(NCH)]

                def flush_list(lst):
                    if not lst:
                        return
                    base0 = lst[0][0]
                    span = lst[-1][0] + lst[-1][1] - base0
                    ps = lst[0][2]
                    lst.clear()
                    nc.scalar.activation(sc[:, base0:base0 + span],
                                         c_sb[:, base0:base0 + span], AF.Tanh)
                    nc.vector.tensor_tensor(
                        xh[:, 1, base0:base0 + span],
                        sifo[:, ps + 2 * NWORD + base0:
                             ps + 2 * NWORD + base0 + span],
                        sc[:, base0:base0 + span], ALU.mult)

                def finalize(host_grp, dbase, c4lo, c4hi):
                    # frozen columns are final: transpose [H,word]->[word,H]
                    # via the PE into the hosting group's g-bank (free once
                    # its sigmoid has read it), one merged PSUM->SBUF copy,
                    # then scatter rows in original word order.
                    gf16 = host_grp[:].bitcast(F16)
                    for c4 in range(c4lo, c4hi):
                        col = dbase + c4 * 128
                        po = 2 * CG + c4 * 128
                        nc.tensor.matmul(
                            gf16[:, po:po + 128], c_sb[:, col:col + 128],
                            ident[:], is_transpose=True,
                            skip_group_check=True)
                    cn = cnpool.tile([128, 512], F16, tag="cn", name="cn")
                    nc.vector.tensor_copy(
                        cn[:, c4lo * 128:c4hi * 128],
                        gf16[:, 2 * CG + c4lo * 128:2 * CG + c4hi * 128])
                    for c4 in range(c4lo, c4hi):
                        cc = (dbase + c4 * 128) // 128
                        nc.gpsimd.indirect_dma_start(
                            out=c_dram[:],
                            out_offset=bass.IndirectOffsetOnAxis(
                                ap=sci_sb[:, cc:cc + 1], axis=0),
                            in_=cn[:, c4 * 128:(c4 + 1) * 128],
                            in_offset=None,
                        )

                def finalize_step(t, host_grp):
                    chunks = [c for c in range(NCH) if fstep[c] == t]
                    for blk in sorted({c // 4 for c in chunks}):
                        cs = [c for c in chunks if c // 4 == blk]
                        finalize(host_grp, blk * BLK,
                                 min(cs) - 4 * blk, max(cs) + 1 - 4 * blk)

                # chunks of block 0 past W[L-1] freeze by step L-2 in
                # every core and can be scattered one step early
                make_identity(nc, ident[:])
                for t in range(L):
                    w = Wx[t]
                    wn = Wx[t + 1]
                    if w == 0:
                        continue
                    nb = (w + BLK - 1) // BLK
                    # leftover tanh(c)+h from the previous step: defer past
                    # this step's first block unless its columns overlap
                    carry = list(oready)
                    oready.clear()
                    if carry and carry[0][0] < BLK:
                        flush_list(carry)
                    if t > 0:  # t=0 one-hot is prefetched before the loop
                        for k in range(nb):
                            a, b = k * BLK, min(w, (k + 1) * BLK)
                            nc.sync.dma_start(xh[:, 0, a:b], oh_in[t, :, a:b])
                    ps = (t % 2) * 4 * NWORD
                    for k in range(nb):
                        base = k * BLK
                        wk = min(BLK, w - base)
                        wkn = max(0, min(BLK, wn - base))
                        grp = g4pool.tile([128, 4 * BLK], F32, tag="g4", name="g4")
                        # gate matmuls; i,f,o first so sigmoid starts early
                        # (no f at t=0 since c0=0; no o for dying blocks).
                        # The g matmul lands in a bank adjacent to the live
                        # gates so one strided sigmoid covers everything.
                        gates = [(0, CI, wk)]
                        if t > 0:
                            gates.append((1, CF, wk))
                        if wkn > 0:
                            gates.append((3, CO, wkn))
                            gcol, gslot = CG, 3
                        elif t > 0:
                            gcol, gslot = CO, 2
                        else:
                            gcol, gslot = CF, 1
                        gates.append((2, gcol, wk))
                        for m, col, gw in gates:
                            if t == 0:
                                # h is uninitialized: one-hot k-tile only
                                nc.tensor.matmul(
                                    grp[:, col:col + gw],
                                    lhsT=lt_sb[:, 0, GS[m]],
                                    rhs=xh[:, 0, base:base + gw],
                                    start=True, stop=not RES)
                                if RES:
                                    nc.tensor.matmul(
                                        grp[:, col:col + gw],
                                        lhsT=ltr_sb[:, 0, GS[m]],
                                        rhs=xh[:, 0, base:base + gw],
                                        start=False, stop=True)
                            else:
                                nc.tensor.matmul(
                                    grp[:, col:col + gw],
                                    lhsT=lt_sb[:, :, GS[m]],
                                    rhs=xh[:, :, base:base + gw],
                                    start=True, stop=not RES, perf_mode=DR)
                                if RES:
                                    nc.tensor.matmul(
                                        grp[:, col:col + gw],
                                        lhsT=ltr_sb[:, :, GS[m]],
                                        rhs=xh[:, :, base:base + gw],
                                        start=False, stop=True, perf_mode=DR)
                        # one strided sigmoid covers every live gate bank
                        if t == 0 and wkn > 0:      # i@0; o@1024,g@1536
                            nc.scalar.activation(sifo[:, ps + base:
                                                      ps + base + wk],
                                                 grp[:, 0:wk], AF.Sigmoid)
                            nc.scalar.activation(
                                view(sifo[:], ps + 2 * NWORD + base,
                                     [[NWORD, 2], [1, wk]]),
                                view(grp[:], CO, [[512, 2], [1, wk]]),
                                AF.Sigmoid)
                        elif t == 0:                # i@0, g@512
                            nc.scalar.activation(
                                view(sifo[:], ps + base, [[NWORD, 2], [1, wk]]),
                                view(grp[:], 0, [[512, 2], [1, wk]]),
                                AF.Sigmoid)
                        elif wkn > 0:               # i,f,o,g @ stride 512
                            nc.scalar.activation(
                                view(sifo[:], ps + base, [[NWORD, 4], [1, wk]]),
                                view(grp[:], 0, [[512, 4], [1, wk]]),
                                AF.Sigmoid)
                        else:                       # i,f,g @ stride 512
                            nc.scalar.activation(
                                view(sifo[:], ps + base, [[NWORD, 3], [1, wk]]),
                                view(grp[:], 0, [[512, 3], [1, wk]]),
                                AF.Sigmoid)
                        # DVE: G = 2*sig(2g)-1 = tanh(g), then c = f*c + i*G
                        gs = sifo[:, ps + gslot * NWORD + base:
                                  ps + gslot * NWORD + base + wk]
                        nc.vector.tensor_scalar(gs, gs, 2.0, 1.0,
                                                ALU.mult, ALU.subtract)
                        if t == 0:
                            nc.vector.tensor_tensor(
                                c_sb[:, base:base + wk],
                                sifo[:, ps + base:ps + base + wk],
                                gs, ALU.mult)
                        else:
                            nc.vector.tensor_tensor(
                                sg[:, base:base + wk],
                                sifo[:, ps + base:ps + base + wk],
                                gs, ALU.mult)
                            nc.vector.tensor_tensor(
                                c_sb[:, base:base + wk],
                                sifo[:, ps + NWORD + base:
                                     ps + NWORD + base + wk],
                                c_sb[:, base:base + wk], ALU.mult)
                            nc.vector.tensor_tensor(
                                c_sb[:, base:base + wk], sg[:, base:base + wk],
                                c_sb[:, base:base + wk], ALU.add)
                        if wkn > 0:
                            oready.append((base, wkn, ps))
                            # narrow steps: flush immediately so the next
                            # step's h dependency clears while the sigmoid
                            # of the later block still runs
                            if len(oready) == 2 or nb <= 2:
                                flush_list(oready)
                        if k == 0:
                            flush_list(carry)
                    finalize_step(t, grp)
                flush_list(oready)
                fgrp = g4pool.tile([128, 4 * BLK], F32, tag="g4", name="g4")
                finalize_step(L - 1, fgrp)
            # ------------------------------------------------------- tail
            with (
                tc.tile_pool(name="tpsum", bufs=2, space="PSUM") as tpsum,
                tc.tile_pool(name="cpsum", bufs=2, space="PSUM") as cpsum,
                tc.tile_pool(name="small", bufs=1) as small,
            ):
                A = small.tile([128, NWORD], F16, tag="A", name="A")
                # A[p, (i*NEC+ec)*128 + h] = c_dram[ec*512 + p*4 + i, h]
                # split by word-index i into 4 parallel DMAs
                geng = [nc.sync, nc.scalar, nc.gpsimd, nc.sync]
                for i4 in range(NW):
                    srcap = bass.AP(
                        c_dram[:].tensor, i4 * H,
                        [[NW * H, 128], [BLK * H, NEC], [1, H]])
                    dst = bass.AP(
                        A.tensor, A.offset + i4 * NEC * 128,
                        [A.ap[0], [128, NEC], [1, H]])
                    geng[i4].dma_start(dst, srcap)

                WSEG = NEC * 128
                prod = small.tile([128, NWORD], F16, tag="prod", name="prod")
                D0 = small.tile([128, NW * NEC], F32, tag="D0", name="D0")
                S = small.tile([128, NW * NEC], F32, tag="S", name="S")
                C6 = small.tile([128, 6 * NEC], F32, tag="C6", name="C6")
                C6h = small.tile([128, 6 * NEC], F16, tag="C6h", name="C6h")
                SS = small.tile([128, 6 * NEC], F32, tag="SS", name="SS")
                cos6 = small.tile([6, PER], F16, tag="cos6", name="cos6")

                # self-dots first (DVE) so the rsqrt chain overlaps the
                # pair products
                for i in range(NW):
                    nc.vector.tensor_tensor(
                        prod[:, i * WSEG:(i + 1) * WSEG],
                        A[:, i * WSEG:(i + 1) * WSEG],
                        A[:, i * WSEG:(i + 1) * WSEG], ALU.mult)
                nc.vector.tensor_reduce(
                    D0[:],
                    prod[:].rearrange("p (i e h) -> p (i e) h", i=NW, e=NEC),
                    axis=mybir.AxisListType.X, op=ALU.add)
                nc.vector.tensor_scalar_max(D0[:], D0[:], 1e-30)
                nc.scalar.activation(S[:], D0[:], AF.Ln)
                nc.scalar.activation(S[:], S[:], AF.Exp, scale=-0.5)
                for kp, (i, j) in enumerate(P6):
                    nc.vector.tensor_tensor(
                        prod[:, :WSEG], A[:, i * WSEG:(i + 1) * WSEG],
                        A[:, j * WSEG:(j + 1) * WSEG], ALU.mult)
                    nc.vector.tensor_reduce(
                        C6[:, kp * NEC:(kp + 1) * NEC],
                        prod[:, :WSEG].rearrange("p (e h) -> p e h", e=NEC),
                        axis=mybir.AxisListType.X, op=ALU.add)
                for kp, (i, j) in enumerate(P6):
                    nc.vector.tensor_tensor(
                        SS[:, kp * NEC:(kp + 1) * NEC],
                        S[:, i * NEC:(i + 1) * NEC],
                        S[:, j * NEC:(j + 1) * NEC], ALU.mult)
                nc.vector.tensor_tensor(C6h[:], C6[:], SS[:], ALU.mult)
                r1 = small.tile([36, PER], F16, tag="r1", name="r1")
                r2 = small.tile([32, PER], F16, tag="r2", name="r2")
                o_sb = small.tile([1, PER], F32, tag="o", name="o")

                def transp(ecs):
                    for ec in ecs:
                        pt_ = tpsum.tile([128, 128], F16, tag="tc", name="tc")
                        cview = bass.AP(C6h.tensor, C6h.offset + ec,
                                        [C6h.ap[0], [NEC, 6]])
                        nc.tensor.transpose(pt_[:6, :], cview, ident[:])
                        nc.vector.tensor_copy(
                            cos6[:, ec * 128:(ec + 1) * 128], pt_[:6, :])

                # two example-halves pipelined through the CNN stages so
                # each ACT stage overlaps the other half's matmuls
                sl = [slice(0, 512), slice(512, 1024)]
                transp(range(0, NEC // 2))
                p1 = [None, None]
                p2 = [None, None]
                p3 = [None, None]
                p1[0] = cpsum.tile([36, 512], F32, tag="cp1", name="cp1")
                nc.tensor.matmul(p1[0][:], lhsT=w1_sb[:], rhs=cos6[:, sl[0]],
                                 start=True, stop=True)
                transp(range(NEC // 2, NEC))
                p1[1] = cpsum.tile([36, 512], F32, tag="cp1", name="cp1")
                nc.tensor.matmul(p1[1][:], lhsT=w1_sb[:], rhs=cos6[:, sl[1]],
                                 start=True, stop=True)
                for h in range(2):
                    nc.scalar.activation(r1[:, sl[h]], p1[h][:], AF.Relu,
                                         bias=b1_sb[:, 0:1])
                    p2[h] = cpsum.tile([32, 512], F32, tag="cp1", name="cp1")
                    nc.tensor.matmul(p2[h][:], lhsT=w2_sb[:], rhs=r1[:, sl[h]],
                                     start=True, stop=True)
                for h in range(2):
                    nc.scalar.activation(r2[:, sl[h]], p2[h][:], AF.Relu,
                                         bias=b2_sb[:, 0:1])
                    p3[h] = cpsum.tile([1, 512], F32, tag="cp1", name="cp1")
                    nc.tensor.matmul(p3[h][:], lhsT=wsc_sb[:], rhs=r2[:, sl[h]],
                                     start=True, stop=True)
                for h in range(2):
                    nc.scalar.activation(o_sb[:, sl[h]], p3[h][:], AF.Sigmoid,
                                         bias=bsc_sb[0:1, 0:1])
                nc.sync.dma_start(out_d[:], o_sb[:])

    return nc


_prog_cache = {}


def _get_program(W):
    key = tuple(int(x) for x in W)
    if key not in _prog_cache:
        _prog_cache[key] = _build_program(key)
    return _prog_cache[key]


def _run(inputs, trace=False):
    consts = _build_consts(inputs)
    word_ids = np.asarray(inputs["word_ids"])
    lengths = np.asarray(inputs["lengths"])

    preps = []
    for c in range(NCORES):
        sl = slice(c * PER, (c + 1) * PER)
        preps.append(_core_prep(word_ids[sl], lengths[sl]))
    Nt_max = np.stack([p[2] for p in preps]).max(0)
    W = tuple(int(min(NWORD, -(-int(n) // 16) * 16)) for n in Nt_max)

    lt_f8 = consts["LT"].astype(F8NP)
    ltr_f8 = consts["LTR"].astype(F8NP)
    in_maps = []
    for c in range(NCORES):
        wid_s, lens_s, _, perm = preps[c]
        in_maps.append({
            "oh": _build_onehot(wid_s, lens_s, W),
            "scidx": _build_scidx(perm),
            "lt": lt_f8, "ltr": ltr_f8,
            "w1": consts["W1eff"].astype(np.float16),
            "b1": consts["b1eff"],
            "w2": consts["W2eff"].astype(np.float16),
            "b2": consts["b2eff"],
            "wsc": consts["Wsc"].astype(np.float16),
            "bsc": np.full((1, 1), consts["bsc"], np.float32),
        })

    nc = _get_program(W)
    _spill_excess_waits(nc)  # idempotent; HW-compile only
    res = run_bass_kernel_spmd(nc, in_maps, list(range(NCORES)), trace=trace)
    out = np.concatenate([np.asarray(r["out"]).reshape(PER) for r in res.results])
    return out.reshape(B, 1).astype(np.float32), res.exec_time_ns


def kernel(**inputs):
    return _run(inputs)[0]


# revision 47
# speedup vs baseline: 1.0021x; 1.0021x over previous
"""Trainium2 Bass kernel for the char-LSTM word-similarity CNN scorer.

Problem: B=8192 examples x NW=4 words x L=16 chars. Per word: char
embeddings -> masked LSTMCell over <=16 steps -> cell state c [128].
Per example: 4x4 cosine matrix of the word reps -> 2-layer 2x2-valid
CNN -> linear scorer -> sigmoid.

Strategy (pure data parallel, 1024 examples / 4096 words per core):
 - Host folds emb @ W_ih.T + (b_ih + b_hh) into a [128, 512] table
   (row 64 = "freeze" flag driving f->1, i->0 for words past their
   length); per-step char inputs are a one-hot matrix.
 - Words sorted by length (desc) on host; step t processes exactly the
   active width N_t (rounded to 16), in 512-column PSUM blocks.
 - fp8e4 DoubleRow matmuls: each gate's pre-activation is ONE PE
   matmul with two k-tiles -- (one-hot @ G65) + (h @ WhhT) -- at 0.5
   cycles per output column (plus an optional residual-table pass that
   cancels the fp8 weight quantization error). The one-hot operand is
   exact in fp8; h is stored as fp8e4 in the same SBUF tile as the
   one-hot so both k-tiles come from a single strided access pattern.
 - ACT work minimized: ALL FOUR gate activations run as ONE strided-AP
   sigmoid per block (g-gate table rows are pre-doubled so tanh(g) =
   2*sigmoid(2g)-1; the fixup is a cheap DVE tensor_scalar);
   o-gate/tanh(c)/h only computed for words still alive at the next
   step; no f-gate at t=0 (c0=0); no memsets (c=i*g at t=0, h never
   read at t=0). The sigmoid output buffer is double-buffered by step
   parity so it never write-after-read stalls against DVE.
 - PSUM: 4-bank gate groups (i|f|o|g), double buffered. Dying blocks
   park g in the o-bank so their own g-bank is a dependency-free PE
   transpose target.
 - As each 512-word block freezes, its c columns are transposed
   ([H,word]->[word,H]) and indirect-DMA scattered to DRAM in original
   word order -- overlapped with the remaining LSTM steps. Chunks of
   the last block past W[15] are provably frozen a step early and
   pre-scattered. The tail is then: one contiguous gather, all 10 dot
   products (6 pairs + 4 self-dots) on DVE, rsqrt norm scaling, and
   the tiny CNN/scorer lowered to matmuls pipelined in two
   example-halves.
"""

import os
import sys

for _p in ("/opt/trn_rl_repo",):
    if _p not in sys.path and os.path.isdir(_p):
        sys.path.insert(0, _p)

import numpy as np
import ml_dtypes

import concourse.bass as bass
import concourse.mybir as mybir
import concourse.tile as tile
from concourse.bass_utils import run_bass_kernel_spmd
from concourse.masks import make_identity

F8NP = ml_dtypes.float8_e4m3

# This container's walrus build rejects CTRL instructions (Drain) carrying
# more than 2 sync waits ("Too many sync wait commands" in setupSyncWait).
# Tile's kernel-tail drain accumulates one wait per engine/DMA-queue sem, so
# redistribute: keep one wait on the drain, move the rest onto nofuse NOPs
# that execute before the all-engine barrier. Semantics are unchanged (all
# waits still complete before the barrier / semaphore teardown).
def _patched_drain_and_barrier(self, tick_clock, wait_clock):
    nc = self.nc
    drain_inst = nc.sync.drain()
    wait_clock.add_sem_waits(
        drain_inst.ins, tile.ScopedClock({None: tick_clock.global_clock})
    )
    waits = list(drain_inst.ins.sync_info.on_wait)
    if len(waits) > 1:
        drain_inst.ins.sync_info.on_wait = waits[:1]
        for k in range(1, len(waits)):
            nop = nc.sync.nop(nofuse=True, hint="drain_wait_spill")
            if nop.ins.sync_info is None:
                nop.ins.sync_info = mybir.SyncInfo(on_wait=[], on_update=[])
            nop.ins.sync_info.on_wait = [waits[k]]
    nc.all_engine_barrier()
    assert self.sems is not None
    popped = nc._tile_sem_poison_stack.pop()
    assert popped is self._sem_poison
    nc.clear_and_free_semaphores(list(self.sems.allocated().values()))
    nc.all_engine_barrier()


tile.TileContext._drain_and_barrier = _patched_drain_and_barrier

def _spill_excess_waits(nc):
    """Walrus here rejects instructions with more than ~2 sync waits. Spill
    excess waits onto same-engine NoOps inserted just before the instruction
    (engines dispatch in program order, so waiting earlier on the same engine
    is equivalent)."""
    cnt = [0]
    for fn in nc.m.functions:
        for bb in fn.blocks:
            insts = list(bb.instructions)
            out = []
            changed = False
            for inst in insts:
                si = inst.sync_info
                waits = list(si.on_wait) if si is not None and si.on_wait else []
                max_waits = 1
                if len(waits) > max_waits:
                    changed = True
                    keep = waits[-max_waits:]
                    extra = waits[:-max_waits]
                    for j in range(0, len(extra), max_waits):
                        cnt[0] += 1
                        nop = mybir.InstNoOp(name=f"I-spillw-{cnt[0]}", ins=[], outs=[])
                        nop.engine = inst.engine
                        nop.sync_info = mybir.SyncInfo(
                            on_wait=extra[j:j + max_waits], on_update=[])
                        nop.bass_nofuse = True
                        nop.bass_priority = 0
                        nop.text_hint = "spillw"
                        nop.debug = inst.debug
                        out.append(nop)
                    si.on_wait = keep
                out.append(inst)
            if changed:
                bb.instructions = out

B, NW, L, E, H, V = 8192, 4, 16, 128, 128, 64
NCORES = 8
PER = B // NCORES          # 1024 examples per core
NWORD = PER * NW           # 4096 words per core
NEC = PER // 128           # 8 example-chunks of 128
BLK = 512
NBLK = NWORD // BLK
FB = 30.0                  # freeze bias magnitude
RES = False                # second DoubleRow pass with fp8 residual tables
F32 = mybir.dt.float32
F16 = mybir.dt.float16
F8 = mybir.dt.float8e4
I32 = mybir.dt.int32
AF = mybir.ActivationFunctionType
ALU = mybir.AluOpType
DR = mybir.MatmulPerfMode.DoubleRow

P6 = [(0, 1), (0, 2), (0, 3), (1, 2), (1, 3), (2, 3)]


# ----------------------------------------------------------------- host prep

def _f8rt(x):
    return x.astype(F8NP).astype(np.float32)


def _build_consts(inp):
    emb = np.asarray(inp["emb_i"], np.float32)
    W_ih = np.asarray(inp["W_ih"], np.float32)
    W_hh = np.asarray(inp["W_hh"], np.float32)
    b = np.asarray(inp["b_ih"], np.float32) + np.asarray(inp["b_hh"], np.float32)
    G65 = np.zeros((128, 4 * H), np.float32)
    G65[:V] = emb @ W_ih.T + b
    G65[V, 0:H] = -FB
    G65[V, H:2 * H] = +FB
    WhhT = np.ascontiguousarray(W_hh.T)  # [H, 4H]
    # g-gate pre-activations are doubled so tanh(g) can be evaluated on the
    # sigmoid table together with i,f,o in one strided ACT instruction:
    # tanh(x) = 2*sigmoid(2x) - 1 (the 2s-1 fixup runs on DVE)
    G65[:, 2 * H:3 * H] *= 2.0
    WhhT[:, 2 * H:3 * H] *= 2.0

    # interleaved DoubleRow lhsT tables [128, 2, 4H]: k-tile 0 = G65
    # (one-hot side), k-tile 1 = WhhT (h side); plus fp8 residuals.
    LT = np.zeros((128, 2, 4 * H), np.float32)
    LTR = np.zeros((128, 2, 4 * H), np.float32)
    LT[:, 0, :] = _f8rt(G65)
    LT[:, 1, :] = _f8rt(WhhT)
    LTR[:, 0, :] = _f8rt(G65 - LT[:, 0, :])
    LTR[:, 1, :] = _f8rt(WhhT - LT[:, 1, :])

    w1 = np.asarray(inp["conv1_w"], np.float32)
    b1 = np.asarray(inp["conv1_b"], np.float32)
    w2 = np.asarray(inp["conv2_w"], np.float32)
    b2 = np.asarray(inp["conv2_b"], np.float32)
    ws = np.asarray(inp["scorer_w"], np.float32)
    bs = float(np.asarray(inp["scorer_b"], np.float32)[0])

    p6idx = {p: i for i, p in enumerate(P6)}
    W1eff = np.zeros((6, 36), np.float32)
    b1eff = np.zeros((36, 1), np.float32)
    for c in range(4):
        for y in range(3):
            for x in range(3):
                m = c * 9 + y * 3 + x
                b1eff[m, 0] += b1[c]
                for dy in range(2):
                    for dx in range(2):
                        a, bb = y + dy, x + dx
                        w = w1[c, 0, dy, dx]
                        if a == bb:
                            b1eff[m, 0] += w
                        else:
                            W1eff[p6idx[(min(a, bb), max(a, bb))], m] += w
    W2eff = np.zeros((36, 32), np.float32)
    b2eff = np.zeros((32, 1), np.float32)
    for c2 in range(8):
        for y in range(2):
            for x in range(2):
                m = c2 * 4 + y * 2 + x
                b2eff[m, 0] = b2[c2]
                for c1 in range(4):
                    for dy in range(2):
                        for dx in range(2):
                            W2eff[c1 * 9 + (y + dy) * 3 + (x + dx), m] += w2[c2, c1, dy, dx]
    Wsc = ws[0].astype(np.float32).reshape(32, 1)
    return dict(LT=LT, LTR=LTR, W1eff=W1eff, b1eff=b1eff,
                W2eff=W2eff, b2eff=b2eff, Wsc=Wsc, bsc=bs)


def _core_prep(word_ids_c, lengths_c):
    wid = np.asarray(word_ids_c).reshape(NWORD, L)
    lens = np.asarray(lengths_c).reshape(NWORD)
    perm = np.argsort(-lens, kind="stable").astype(np.int32)
    wid_s = wid[perm]
    lens_s = lens[perm]
    Nt = (np.arange(L)[:, None] < lens_s[None, :]).sum(1)  # [L]
    return wid_s, lens_s, Nt, perm


def _build_onehot(wid_s, lens_s, widths):
    oh = np.zeros((L, 128, NWORD), F8NP)
    one = np.ones((), F8NP)
    cols = np.arange(NWORD)
    for t in range(L):
        n = int(widths[t])
        if n == 0:
            continue
        alive = lens_s[:n] > t
        oh[t, wid_s[:n, t], cols[:n]] = np.where(alive, one, np.zeros((), F8NP))
        oh[t, V, cols[:n]] = np.where(alive, np.zeros((), F8NP), one)
    return oh


def _build_scidx(perm):
    # scidx[p, cc] = original index of sorted word cc*128+p
    return np.ascontiguousarray(
        perm.reshape(NWORD // 128, 128).T).astype(np.int32)


# -------------------------------------------------------------- bass program

def _build_program(W):
    """W: tuple of per-step active widths (len 16, multiples of 16, desc)."""
    nc = bass.Bass()
    Wx = list(W) + [0]

    oh_in = nc.dram_tensor("oh", [L, 128, NWORD], F8, kind="ExternalInput")
    sci_in = nc.dram_tensor("scidx", [128, NWORD // 128], I32, kind="ExternalInput")
    lt_in = nc.dram_tensor("lt", [128, 2, 4 * H], F8, kind="ExternalInput")
    ltr_in = nc.dram_tensor("ltr", [128, 2, 4 * H], F8, kind="ExternalInput")
    w1_in = nc.dram_tensor("w1", [6, 36], F16, kind="ExternalInput")
    b1_in = nc.dram_tensor("b1", [36, 1], F32, kind="ExternalInput")
    w2_in = nc.dram_tensor("w2", [36, 32], F16, kind="ExternalInput")
    b2_in = nc.dram_tensor("b2", [32, 1], F32, kind="ExternalInput")
    wsc_in = nc.dram_tensor("wsc", [32, 1], F16, kind="ExternalInput")
    bsc_in = nc.dram_tensor("bsc", [1, 1], F32, kind="ExternalInput")
    out_d = nc.dram_tensor("out", [1, PER], F32, kind="ExternalOutput")
    c_dram = nc.dram_tensor("cscratch", [NWORD, H], F16)

    def view(ap, off, dims):
        return bass.AP(ap.tensor, ap.offset + off, [ap.ap[0]] + dims)

    with tile.TileContext(nc) as tc:
        with (
            tc.tile_pool(name="const", bufs=1) as cpool,
            tc.tile_pool(name="state", bufs=1) as spool,
        ):
            lt_sb = cpool.tile([128, 2, 4 * H], F8, tag="lt", name="lt")
            ltr_sb = cpool.tile([128, 2, 4 * H], F8, tag="ltr", name="ltr") \
                if RES else None
            w1_sb = cpool.tile([6, 36], F16, tag="w1", name="w1")
            b1_sb = cpool.tile([36, 1], F32, tag="b1", name="b1")
            w2_sb = cpool.tile([36, 32], F16, tag="w2", name="w2")
            b2_sb = cpool.tile([32, 1], F32, tag="b2", name="b2")
            wsc_sb = cpool.tile([32, 1], F16, tag="wsc", name="wsc")
            bsc_sb = cpool.tile([1, 1], F32, tag="bsc", name="bsc")
            ident = cpool.tile([128, 128], F16, tag="ident", name="ident")
            sci_sb = cpool.tile([128, NWORD // 128], I32, tag="sci", name="sci")

            # xh: slot 0 = per-step one-hot (DMA), slot 1 = h state (fp8)
            xh = spool.tile([128, 2, NWORD], F8, tag="xh", name="xh")
            c_sb = spool.tile([128, NWORD], F16, tag="c", name="c")
            # sifo slots: 0=sig(i) 1=sig(f) 2=sig(o) 3=sig(2g), double
            # buffered by step parity so step t+1's sigmoid never waits for
            # DVE's reads of step t's values (write-after-read)
            sifo = spool.tile([128, 8 * NWORD], F16, tag="sifo", name="sifo")
            sg = spool.tile([128, NWORD], F16, tag="sg", name="sg")
            sc = spool.tile([128, NWORD], F16, tag="sc", name="sc")

            # startup: the critical first-matmul inputs (lt + step-0 one-hot)
            # issue first, spread over engine queues that are idle at t=0 so
            # the ~900ns-per-issue DMA costs overlap. Everything else follows
            # on sync/gpsimd.
            nc.sync.dma_start(lt_sb[:], lt_in[:])
            w0 = Wx[0]
            nb0 = (w0 + BLK - 1) // BLK
            eng0 = [nc.scalar, nc.gpsimd, nc.sync, nc.scalar,
                    nc.gpsimd, nc.sync, nc.scalar, nc.gpsimd]
            for k in range(nb0):
                a, b = k * BLK, min(w0, (k + 1) * BLK)
                eng0[k].dma_start(xh[:, 0, a:b], oh_in[0, :, a:b])
            consts = [(w1_sb, w1_in), (b1_sb, b1_in), (w2_sb, w2_in),
                      (b2_sb, b2_in), (wsc_sb, wsc_in), (bsc_sb, bsc_in),
                      (sci_sb, sci_in)]
            if RES:
                consts.append((ltr_sb, ltr_in))
            for sb, dr in consts:
                nc.gpsimd.dma_start(sb[:], dr[:])

            # ------------------------------------------------ LSTM main loop
            with (
                tc.tile_pool(name="g4", bufs=2, space="PSUM") as g4pool,
                tc.tile_pool(name="cn", bufs=4) as cnpool,
            ):
                # gate column offsets inside a [128, 2048] 4-bank group
                CI, CF, CO, CG = 0, 512, 1024, 1536
                GS = {0: slice(0, H), 1: slice(H, 2 * H),
                      2: slice(2 * H, 3 * H), 3: slice(3 * H, 4 * H)}
                oready = []  # [(base, wkn)] tanh(c)+h pending
                # chunk c (cols [128c, 128c+128)) is provably frozen in
                # every core once the next step's width cannot reach it;
                # finalize (transpose+scatter) each chunk at that step so
                # the WAW-serialized scatter chain drains early
                NCH = NWORD // 128
                fstep = [min(t for t in range(L)
                             if Wx[t + 1] <= 128 * c) for c in range(NCH)]

                def flush_list(lst):
                    if not lst:
                        return
                    base0 = lst[0][0]
                    span = lst[-1][0] + lst[-1][1] - base0
                    ps = lst[0][2]
                    lst.clear()
                    nc.scalar.activation(sc[:, base0:base0 + span],
                                         c_sb[:, base0:base0 + span], AF.Tanh)
                    nc.vector.tensor_tensor(
                        xh[:, 1, base0:base0 + span],
                        sifo[:, ps + 2 * NWORD + base0:
                             ps + 2 * NWORD + base0 + span],
                        sc[:, base0:base0 + span], ALU.mult)

                def finalize(host_grp, dbase, c4lo, c4hi):
                    # frozen columns are final: transpose [H,word]->[word,H]
                    # via the PE into the hosting group's g-bank (free once
                    # its sigmoid has read it), one merged PSUM->SBUF copy,
                    # then scatter rows in original word order.
                    gf16 = host_grp[:].bitcast(F16)
                    for c4 in range(c4lo, c4hi):
                        col = dbase + c4 * 128
                        po = 2 * CG + c4 * 128
                        nc.tensor.matmul(
                            gf16[:, po:po + 128], c_sb[:, col:col + 128],
                            ident[:], is_transpose=True,
                            skip_group_check=True)
                    cn = cnpool.tile([128, 512], F16, tag="cn", name="cn")
                    nc.vector.tensor_copy(
                        cn[:, c4lo * 128:c4hi * 128],
                        gf16[:, 2 * CG + c4lo * 128:2 * CG + c4hi * 128])
                    for c4 in range(c4lo, c4hi):
                        cc = (dbase + c4 * 128) // 128
                        nc.gpsimd.indirect_dma_start(
                            out=c_dram[:],
                            out_offset=bass.IndirectOffsetOnAxis(
                                ap=sci_sb[:, cc:cc + 1], axis=0),
                            in_=cn[:, c4 * 128:(c4 + 1) * 128],
                            in_offset=None,
                        )

                def finalize_step(t, host_grp):
                    # blocks >= 2 are finalized whole at death (below);
                    # only the tail-critical chunks of blocks 0-1 use the
                    # per-chunk earliest-freeze schedule
                    chunks = [c for c in range(8) if fstep[c] == t]
                    for blk in sorted({c // 4 for c in chunks}):
                        cs = [c for c in chunks if c // 4 == blk]
                        finalize(host_grp, blk * BLK,
                                 min(cs) - 4 * blk, max(cs) + 1 - 4 * blk)

                # chunks of block 0 past W[L-1] freeze by step L-2 in
                # every core and can be scattered one step early
                make_identity(nc, ident[:])
                for t in range(L):
                    w = Wx[t]
                    wn = Wx[t + 1]
                    if w == 0:
                        continue
                    nb = (w + BLK - 1) // BLK
                    # leftover tanh(c)+h from the previous step: defer past
                    # this step's first block unless its columns overlap
                    carry = list(oready)
                    oready.clear()
                    if carry and carry[0][0] < BLK:
                        flush_list(carry)
                    if t > 0:  # t=0 one-hot is prefetched before the loop
                        for k in range(nb):
                            a, b = k * BLK, min(w, (k + 1) * BLK)
                            nc.sync.dma_start(xh[:, 0, a:b], oh_in[t, :, a:b])
                    ps = (t % 2) * 4 * NWORD
                    for k in range(nb):
                        base = k * BLK
                        wk = min(BLK, w - base)
                        wkn = max(0, min(BLK, wn - base))
                        grp = g4pool.tile([128, 4 * BLK], F32, tag="g4", name="g4")
                        # gate matmuls; i,f,o first so sigmoid starts early
                        # (no f at t=0 since c0=0; no o for dying blocks).
                        # The g matmul lands in a bank adjacent to the live
                        # gates so one strided sigmoid covers everything.
                        gates = [(0, CI, wk)]
                        if t > 0:
                            gates.append((1, CF, wk))
                        if wkn > 0:
                            gates.append((3, CO, wkn))
                            gcol, gslot = CG, 3
                        elif t > 0:
                            gcol, gslot = CO, 2
                        else:
                            gcol, gslot = CF, 1
                        gates.append((2, gcol, wk))
                        for m, col, gw in gates:
                            if t == 0:
                                # h is uninitialized: one-hot k-tile only
                                nc.tensor.matmul(
                                    grp[:, col:col + gw],
                                    lhsT=lt_sb[:, 0, GS[m]],
                                    rhs=xh[:, 0, base:base + gw],
                                    start=True, stop=not RES)
                                if RES:
                                    nc.tensor.matmul(
                                        grp[:, col:col + gw],
                                        lhsT=ltr_sb[:, 0, GS[m]],
                                        rhs=xh[:, 0, base:base + gw],
                                        start=False, stop=True)
                            else:
                                nc.tensor.matmul(
                                    grp[:, col:col + gw],
                                    lhsT=lt_sb[:, :, GS[m]],
                                    rhs=xh[:, :, base:base + gw],
                                    start=True, stop=not RES, perf_mode=DR)
                                if RES:
                                    nc.tensor.matmul(
                                        grp[:, col:col + gw],
                                        lhsT=ltr_sb[:, :, GS[m]],
                                        rhs=xh[:, :, base:base + gw],
                                        start=False, stop=True, perf_mode=DR)
                        # one strided sigmoid covers every live gate bank
                        if t == 0 and wkn > 0:      # i@0; o@1024,g@1536
                            nc.scalar.activation(sifo[:, ps + base:
                                                      ps + base + wk],
                                                 grp[:, 0:wk], AF.Sigmoid)
                            nc.scalar.activation(
                                view(sifo[:], ps + 2 * NWORD + base,
                                     [[NWORD, 2], [1, wk]]),
                                view(grp[:], CO, [[512, 2], [1, wk]]),
                                AF.Sigmoid)
                        elif t == 0:                # i@0, g@512
                            nc.scalar.activation(
                                view(sifo[:], ps + base, [[NWORD, 2], [1, wk]]),
                                view(grp[:], 0, [[512, 2], [1, wk]]),
                                AF.Sigmoid)
                        elif wkn > 0:               # i,f,o,g @ stride 512
                            nc.scalar.activation(
                                view(sifo[:], ps + base, [[NWORD, 4], [1, wk]]),
                                view(grp[:], 0, [[512, 4], [1, wk]]),
                                AF.Sigmoid)
                        else:                       # i,f,g @ stride 512
                            nc.scalar.activation(
                                view(sifo[:], ps + base, [[NWORD, 3], [1, wk]]),
                                view(grp[:], 0, [[512, 3], [1, wk]]),
                                AF.Sigmoid)
                        # DVE: G = 2*sig(2g)-1 = tanh(g), then c = f*c + i*G
                        gs = sifo[:, ps + gslot * NWORD + base:
                                  ps + gslot * NWORD + base + wk]
                        nc.vector.tensor_scalar(gs, gs, 2.0, 1.0,
                                                ALU.mult, ALU.subtract)
                        if t == 0:
                            nc.vector.tensor_tensor(
                                c_sb[:, base:base + wk],
                                sifo[:, ps + base:ps + base + wk],
                                gs, ALU.mult)
                        else:
                            nc.vector.tensor_tensor(
                                sg[:, base:base + wk],
                                sifo[:, ps + base:ps + base + wk],
                                gs, ALU.mult)
                            nc.vector.tensor_tensor(
                                c_sb[:, base:base + wk],
                                sifo[:, ps + NWORD + base:
                                     ps + NWORD + base + wk],
                                c_sb[:, base:base + wk], ALU.mult)
                            nc.vector.tensor_tensor(
                                c_sb[:, base:base + wk], sg[:, base:base + wk],
                                c_sb[:, base:base + wk], ALU.add)
                        if wkn > 0:
                            oready.append((base, wkn, ps))
                            # narrow steps: flush immediately so the next
                            # step's h dependency clears while the sigmoid
                            # of the later block still runs
                            if len(oready) == 2 or nb <= 2:
                                flush_list(oready)
                        elif base >= 2 * BLK:
                            # dying blocks park g in the o-bank, so their
                            # own g-bank is a dependency-free transpose
                            # target; scatters start immediately
                            finalize(grp, base, 0, 4)
                        if k == 0:
                            flush_list(carry)
                    finalize_step(t, grp)
                flush_list(oready)
                fgrp = g4pool.tile([128, 4 * BLK], F32, tag="g4", name="g4")
                finalize_step(L - 1, fgrp)
            # ------------------------------------------------------- tail
            with (
                tc.tile_pool(name="tpsum", bufs=2, space="PSUM") as tpsum,
                tc.tile_pool(name="cpsum", bufs=2, space="PSUM") as cpsum,
                tc.tile_pool(name="small", bufs=1) as small,
            ):
                A = small.tile([128, NWORD], F16, tag="A", name="A")
                # A[p, (i*NEC+ec)*128 + h] = c_dram[ec*512 + p*4 + i, h]
                # split by word-index i into 4 parallel DMAs
                geng = [nc.sync, nc.scalar, nc.gpsimd, nc.sync]
                for i4 in range(NW):
                    srcap = bass.AP(
                        c_dram[:].tensor, i4 * H,
                        [[NW * H, 128], [BLK * H, NEC], [1, H]])
                    dst = bass.AP(
                        A.tensor, A.offset + i4 * NEC * 128,
                        [A.ap[0], [128, NEC], [1, H]])
                    geng[i4].dma_start(dst, srcap)

                WSEG = NEC * 128
                prod = small.tile([128, NWORD], F16, tag="prod", name="prod")
                D0 = small.tile([128, NW * NEC], F32, tag="D0", name="D0")
                S = small.tile([128, NW * NEC], F32, tag="S", name="S")
                C6 = small.tile([128, 6 * NEC], F32, tag="C6", name="C6")
                C6h = small.tile([128, 6 * NEC], F16, tag="C6h", name="C6h")
                SS = small.tile([128, 6 * NEC], F32, tag="SS", name="SS")
                cos6 = small.tile([6, PER], F16, tag="cos6", name="cos6")

                # self-dots first (DVE) so the rsqrt chain overlaps the
                # pair products
                for i in range(NW):
                    nc.vector.tensor_tensor(
                        prod[:, i * WSEG:(i + 1) * WSEG],
                        A[:, i * WSEG:(i + 1) * WSEG],
                        A[:, i * WSEG:(i + 1) * WSEG], ALU.mult)
                nc.vector.tensor_reduce(
                    D0[:],
                    prod[:].rearrange("p (i e h) -> p (i e) h", i=NW, e=NEC),
                    axis=mybir.AxisListType.X, op=ALU.add)
                nc.vector.tensor_scalar_max(D0[:], D0[:], 1e-30)
                nc.scalar.activation(S[:], D0[:], AF.Ln)
                nc.scalar.activation(S[:], S[:], AF.Exp, scale=-0.5)
                for kp, (i, j) in enumerate(P6):
                    nc.vector.tensor_tensor(
                        prod[:, :WSEG], A[:, i * WSEG:(i + 1) * WSEG],
                        A[:, j * WSEG:(j + 1) * WSEG], ALU.mult)
                    nc.vector.tensor_reduce(
                        C6[:, kp * NEC:(kp + 1) * NEC],
                        prod[:, :WSEG].rearrange("p (e h) -> p e h", e=NEC),
                        axis=mybir.AxisListType.X, op=ALU.add)
                for kp, (i, j) in enumerate(P6):
                    nc.vector.tensor_tensor(
                        SS[:, kp * NEC:(kp + 1) * NEC],
                        S[:, i * NEC:(i + 1) * NEC],
                        S[:, j * NEC:(j + 1) * NEC], ALU.mult)
                nc.vector.tensor_tensor(C6h[:], C6[:], SS[:], ALU.mult)
                r1 = small.tile([36, PER], F16, tag="r1", name="r1")
                r2 = small.tile([32, PER], F16, tag="r2", name="r2")
                o_sb = small.tile([1, PER], F32, tag="o", name="o")

                def transp(ecs):
                    for ec in ecs:
                        pt_ = tpsum.tile([128, 128], F16, tag="tc", name="tc")
                        cview = bass.AP(C6h.tensor, C6h.offset + ec,
                                        [C6h.ap[0], [NEC, 6]])
                        nc.tensor.transpose(pt_[:6, :], cview, ident[:])
                        nc.vector.tensor_copy(
                            cos6[:, ec * 128:(ec + 1) * 128], pt_[:6, :])

                # two example-halves pipelined through the CNN stages so
                # each ACT stage overlaps the other half's matmuls
                sl = [slice(0, 512), slice(512, 1024)]
                transp(range(0, NEC // 2))
                p1 = [None, None]
                p2 = [None, None]
                p3 = [None, None]
                p1[0] = cpsum.tile([36, 512], F32, tag="cp1", name="cp1")
                nc.tensor.matmul(p1[0][:], lhsT=w1_sb[:], rhs=cos6[:, sl[0]],
                                 start=True, stop=True)
                transp(range(NEC // 2, NEC))
                p1[1] = cpsum.tile([36, 512], F32, tag="cp1", name="cp1")
                nc.tensor.matmul(p1[1][:], lhsT=w1_sb[:], rhs=cos6[:, sl[1]],
                                 start=True, stop=True)
                for h in range(2):
                    nc.scalar.activation(r1[:, sl[h]], p1[h][:], AF.Relu,
                                         bias=b1_sb[:, 0:1])
                    p2[h] = cpsum.tile([32, 512], F32, tag="cp1", name="cp1")
                    nc.tensor.matmul(p2[h][:], lhsT=w2_sb[:], rhs=r1[:, sl[h]],
                                     start=True, stop=True)
                for h in range(2):
                    nc.scalar.activation(r2[:, sl[h]], p2[h][:], AF.Relu,
                                         bias=b2_sb[:, 0:1])
                    p3[h] = cpsum.tile([1, 512], F32, tag="cp1", name="cp1")
                    nc.tensor.matmul(p3[h][:], lhsT=wsc_sb[:], rhs=r2[:, sl[h]],
                                     start=True, stop=True)
                for h in range(2):
                    nc.scalar.activation(o_sb[:, sl[h]], p3[h][:], AF.Sigmoid,
                                         bias=bsc_sb[0:1, 0:1])
                nc.sync.dma_start(out_d[:], o_sb[:])

    return nc


_prog_cache = {}


def _get_program(W):
    key = tuple(int(x) for x in W)
    if key not in _prog_cache:
        _prog_cache[key] = _build_program(key)
    return _prog_cache[key]


def _run(inputs, trace=False):
    consts = _build_consts(inputs)
    word_ids = np.asarray(inputs["word_ids"])
    lengths = np.asarray(inputs["lengths"])

    preps = []
    for c in range(NCORES):
        sl = slice(c * PER, (c + 1) * PER)
        preps.append(_core_prep(word_ids[sl], lengths[sl]))
    Nt_max = np.stack([p[2] for p in preps]).max(0)
    W = tuple(int(min(NWORD, -(-int(n) // 16) * 16)) for n in Nt_max)

    lt_f8 = consts["LT"].astype(F8NP)
    ltr_f8 = consts["LTR"].astype(F8NP)
    in_maps = []
    for c in range(NCORES):
        wid_s, lens_s, _, perm = preps[c]
        in_maps.append({
            "oh": _build_onehot(wid_s, lens_s, W),
            "scidx": _build_scidx(perm),
            "lt": lt_f8, "ltr": ltr_f8,
            "w1": consts["W1eff"].astype(np.float16),
            "b1": consts["b1eff"],
            "w2": consts["W2eff"].astype(np.float16),
            "b2": consts["b2eff"],
            "wsc": consts["Wsc"].astype(np.float16),
            "bsc": np.full((1, 1), consts["bsc"], np.float32),
        })

    nc = _get_program(W)
    _spill_excess_waits(nc)  # idempotent; HW-compile only
    res = run_bass_kernel_spmd(nc, in_maps, list(range(NCORES)), trace=trace)
    out = np.concatenate([np.asarray(r["out"]).reshape(PER) for r in res.results])
    return out.reshape(B, 1).astype(np.float32), res.exec_time_ns


def kernel(**inputs):
    return _run(inputs)[0]


# revision 48
# speedup vs baseline: 1.0162x; 1.0141x over previous
"""Trainium2 Bass kernel for the char-LSTM word-similarity CNN scorer.

Problem: B=8192 examples x NW=4 words x L=16 chars. Per word: char
embeddings -> masked LSTMCell over <=16 steps -> cell state c [128].
Per example: 4x4 cosine matrix of the word reps -> 2-layer 2x2-valid
CNN -> linear scorer -> sigmoid.

Strategy (pure data parallel, 1024 examples / 4096 words per core):
 - Host folds emb @ W_ih.T + (b_ih + b_hh) into a [128, 512] table
   (row 64 = "freeze" flag driving f->1, i->0 for words past their
   length); per-step char inputs are a one-hot matrix.
 - Words sorted by length (desc) on host; step t processes exactly the
   active width N_t (rounded to 16), in 512-column PSUM blocks.
 - fp8e4 DoubleRow matmuls: each gate's pre-activation is ONE PE
   matmul with two k-tiles -- (one-hot @ G65) + (h @ WhhT) -- at 0.5
   cycles per output column (plus an optional residual-table pass that
   cancels the fp8 weight quantization error). The one-hot operand is
   exact in fp8; h is stored as fp8e4 in the same SBUF tile as the
   one-hot so both k-tiles come from a single strided access pattern.
 - ACT work minimized: ALL FOUR gate activations run as ONE strided-AP
   sigmoid per block (g-gate table rows are pre-doubled so tanh(g) =
   2*sigmoid(2g)-1; the fixup is a cheap DVE tensor_scalar);
   o-gate/tanh(c)/h only computed for words still alive at the next
   step; no f-gate at t=0 (c0=0); no memsets (c=i*g at t=0, h never
   read at t=0). The sigmoid output buffer is double-buffered by step
   parity so it never write-after-read stalls against DVE.
 - PSUM: 4-bank gate groups (i|f|o|g), double buffered. Dying blocks
   park g in the o-bank so their own g-bank is a dependency-free PE
   transpose target.
 - As each 512-word block freezes, its c columns are transposed
   ([H,word]->[word,H]) and indirect-DMA scattered to DRAM in original
   word order -- overlapped with the remaining LSTM steps. Chunks of
   the last block past W[15] are provably frozen a step early and
   pre-scattered. The tail is then: one contiguous gather, all 10 dot
   products (6 pairs + 4 self-dots) on DVE, rsqrt norm scaling, and
   the tiny CNN/scorer lowered to matmuls pipelined in two
   example-halves.
"""

import os
import sys

for _p in ("/opt/trn_rl_repo",):
    if _p not in sys.path and os.path.isdir(_p):
        sys.path.insert(0, _p)

import numpy as np
import ml_dtypes

import concourse.bass as bass
import concourse.mybir as mybir
import concourse.tile as tile
from concourse.bass_utils import run_bass_kernel_spmd
from concourse.masks import make_identity

F8NP = ml_dtypes.float8_e4m3

# This container's walrus build rejects CTRL instructions (Drain) carrying
# more than 2 sync waits ("Too many sync wait commands" in setupSyncWait).
# Tile's kernel-tail drain accumulates one wait per engine/DMA-queue sem, so
# redistribute: keep one wait on the drain, move the rest onto nofuse NOPs
# that execute before the all-engine barrier. Semantics are unchanged (all
# waits still complete before the barrier / semaphore teardown).
def _patched_drain_and_barrier(self, tick_clock, wait_clock):
    nc = self.nc
    drain_inst = nc.sync.drain()
    wait_clock.add_sem_waits(
        drain_inst.ins, tile.ScopedClock({None: tick_clock.global_clock})
    )
    waits = list(drain_inst.ins.sync_info.on_wait)
    if len(waits) > 1:
        drain_inst.ins.sync_info.on_wait = waits[:1]
        for k in range(1, len(waits)):
            nop = nc.sync.nop(nofuse=True, hint="drain_wait_spill")
            if nop.ins.sync_info is None:
                nop.ins.sync_info = mybir.SyncInfo(on_wait=[], on_update=[])
            nop.ins.sync_info.on_wait = [waits[k]]
    nc.all_engine_barrier()
    assert self.sems is not None
    popped = nc._tile_sem_poison_stack.pop()
    assert popped is self._sem_poison
    nc.clear_and_free_semaphores(list(self.sems.allocated().values()))
    nc.all_engine_barrier()


tile.TileContext._drain_and_barrier = _patched_drain_and_barrier

def _spill_excess_waits(nc):
    """Walrus here rejects instructions with more than ~2 sync waits. Spill
    excess waits onto same-engine NoOps inserted just before the instruction
    (engines dispatch in program order, so waiting earlier on the same engine
    is equivalent)."""
    cnt = [0]
    for fn in nc.m.functions:
        for bb in fn.blocks:
            insts = list(bb.instructions)
            out = []
            changed = False
            for inst in insts:
                si = inst.sync_info
                waits = list(si.on_wait) if si is not None and si.on_wait else []
                max_waits = 1
                if len(waits) > max_waits:
                    changed = True
                    keep = waits[-max_waits:]
                    extra = waits[:-max_waits]
                    for j in range(0, len(extra), max_waits):
                        cnt[0] += 1
                        nop = mybir.InstNoOp(name=f"I-spillw-{cnt[0]}", ins=[], outs=[])
                        nop.engine = inst.engine
                        nop.sync_info = mybir.SyncInfo(
                            on_wait=extra[j:j + max_waits], on_update=[])
                        nop.bass_nofuse = True
                        nop.bass_priority = 0
                        nop.text_hint = "spillw"
                        nop.debug = inst.debug
                        out.append(nop)
                    si.on_wait = keep
                out.append(inst)
            if changed:
                bb.instructions = out

B, NW, L, E, H, V = 8192, 4, 16, 128, 128, 64
NCORES = 8
PER = B // NCORES          # 1024 examples per core
NWORD = PER * NW           # 4096 words per core
NEC = PER // 128           # 8 example-chunks of 128
BLK = 512
NBLK = NWORD // BLK
FB = 30.0                  # freeze bias magnitude
RES = False                # second DoubleRow pass with fp8 residual tables
F32 = mybir.dt.float32
F16 = mybir.dt.float16
F8 = mybir.dt.float8e4
I32 = mybir.dt.int32
AF = mybir.ActivationFunctionType
ALU = mybir.AluOpType
DR = mybir.MatmulPerfMode.DoubleRow

P6 = [(0, 1), (0, 2), (0, 3), (1, 2), (1, 3), (2, 3)]


# ----------------------------------------------------------------- host prep

def _f8rt(x):
    return x.astype(F8NP).astype(np.float32)


def _build_consts(inp):
    emb = np.asarray(inp["emb_i"], np.float32)
    W_ih = np.asarray(inp["W_ih"], np.float32)
    W_hh = np.asarray(inp["W_hh"], np.float32)
    b = np.asarray(inp["b_ih"], np.float32) + np.asarray(inp["b_hh"], np.float32)
    G65 = np.zeros((128, 4 * H), np.float32)
    G65[:V] = emb @ W_ih.T + b
    G65[V, 0:H] = -FB
    G65[V, H:2 * H] = +FB
    WhhT = np.ascontiguousarray(W_hh.T)  # [H, 4H]
    # g-gate pre-activations are doubled so tanh(g) can be evaluated on the
    # sigmoid table together with i,f,o in one strided ACT instruction:
    # tanh(x) = 2*sigmoid(2x) - 1 (the 2s-1 fixup runs on DVE)
    G65[:, 2 * H:3 * H] *= 2.0
    WhhT[:, 2 * H:3 * H] *= 2.0

    # interleaved DoubleRow lhsT tables [128, 2, 4H]: k-tile 0 = G65
    # (one-hot side), k-tile 1 = WhhT (h side); plus fp8 residuals.
    LT = np.zeros((128, 2, 4 * H), np.float32)
    LTR = np.zeros((128, 2, 4 * H), np.float32)
    LT[:, 0, :] = _f8rt(G65)
    LT[:, 1, :] = _f8rt(WhhT)
    LTR[:, 0, :] = _f8rt(G65 - LT[:, 0, :])
    LTR[:, 1, :] = _f8rt(WhhT - LT[:, 1, :])

    w1 = np.asarray(inp["conv1_w"], np.float32)
    b1 = np.asarray(inp["conv1_b"], np.float32)
    w2 = np.asarray(inp["conv2_w"], np.float32)
    b2 = np.asarray(inp["conv2_b"], np.float32)
    ws = np.asarray(inp["scorer_w"], np.float32)
    bs = float(np.asarray(inp["scorer_b"], np.float32)[0])

    p6idx = {p: i for i, p in enumerate(P6)}
    W1eff = np.zeros((6, 36), np.float32)
    b1eff = np.zeros((36, 1), np.float32)
    for c in range(4):
        for y in range(3):
            for x in range(3):
                m = c * 9 + y * 3 + x
                b1eff[m, 0] += b1[c]
                for dy in range(2):
                    for dx in range(2):
                        a, bb = y + dy, x + dx
                        w = w1[c, 0, dy, dx]
                        if a == bb:
                            b1eff[m, 0] += w
                        else:
                            W1eff[p6idx[(min(a, bb), max(a, bb))], m] += w
    W2eff = np.zeros((36, 32), np.float32)
    b2eff = np.zeros((32, 1), np.float32)
    for c2 in range(8):
        for y in range(2):
            for x in range(2):
                m = c2 * 4 + y * 2 + x
                b2eff[m, 0] = b2[c2]
                for c1 in range(4):
                    for dy in range(2):
                        for dx in range(2):
                            W2eff[c1 * 9 + (y + dy) * 3 + (x + dx), m] += w2[c2, c1, dy, dx]
    Wsc = ws[0].astype(np.float32).reshape(32, 1)
    return dict(LT=LT, LTR=LTR, W1eff=W1eff, b1eff=b1eff,
                W2eff=W2eff, b2eff=b2eff, Wsc=Wsc, bsc=bs)


def _core_prep(word_ids_c, lengths_c):
    wid = np.asarray(word_ids_c).reshape(NWORD, L)
    lens = np.asarray(lengths_c).reshape(NWORD)
    perm = np.argsort(-lens, kind="stable").astype(np.int32)
    wid_s = wid[perm]
    lens_s = lens[perm]
    Nt = (np.arange(L)[:, None] < lens_s[None, :]).sum(1)  # [L]
    return wid_s, lens_s, Nt, perm


def _build_onehot(wid_s, lens_s, widths):
    oh = np.zeros((L, 128, NWORD), F8NP)
    one = np.ones((), F8NP)
    cols = np.arange(NWORD)
    for t in range(L):
        n = int(widths[t])
        if n == 0:
            continue
        alive = lens_s[:n] > t
        oh[t, wid_s[:n, t], cols[:n]] = np.where(alive, one, np.zeros((), F8NP))
        oh[t, V, cols[:n]] = np.where(alive, np.zeros((), F8NP), one)
    return oh


def _build_scidx(perm):
    # scidx[p, cc] = original index of sorted word cc*128+p
    return np.ascontiguousarray(
        perm.reshape(NWORD // 128, 128).T).astype(np.int32)


# -------------------------------------------------------------- bass program

def _build_program(W):
    """W: tuple of per-step active widths (len 16, multiples of 16, desc)."""
    nc = bass.Bass()
    Wx = list(W) + [0]

    oh_in = nc.dram_tensor("oh", [L, 128, NWORD], F8, kind="ExternalInput")
    sci_in = nc.dram_tensor("scidx", [128, NWORD // 128], I32, kind="ExternalInput")
    lt_in = nc.dram_tensor("lt", [128, 2, 4 * H], F8, kind="ExternalInput")
    ltr_in = nc.dram_tensor("ltr", [128, 2, 4 * H], F8, kind="ExternalInput")
    w1_in = nc.dram_tensor("w1", [6, 36], F16, kind="ExternalInput")
    b1_in = nc.dram_tensor("b1", [36, 1], F32, kind="ExternalInput")
    w2_in = nc.dram_tensor("w2", [36, 32], F16, kind="ExternalInput")
    b2_in = nc.dram_tensor("b2", [32, 1], F32, kind="ExternalInput")
    wsc_in = nc.dram_tensor("wsc", [32, 1], F16, kind="ExternalInput")
    bsc_in = nc.dram_tensor("bsc", [1, 1], F32, kind="ExternalInput")
    out_d = nc.dram_tensor("out", [1, PER], F32, kind="ExternalOutput")
    c_dram = nc.dram_tensor("cscratch", [NWORD, H], F16)

    def view(ap, off, dims):
        return bass.AP(ap.tensor, ap.offset + off, [ap.ap[0]] + dims)

    with tile.TileContext(nc) as tc:
        with (
            tc.tile_pool(name="const", bufs=1) as cpool,
            tc.tile_pool(name="state", bufs=1) as spool,
        ):
            lt_sb = cpool.tile([128, 2, 4 * H], F8, tag="lt", name="lt")
            ltr_sb = cpool.tile([128, 2, 4 * H], F8, tag="ltr", name="ltr") \
                if RES else None
            w1_sb = cpool.tile([6, 36], F16, tag="w1", name="w1")
            b1_sb = cpool.tile([36, 1], F32, tag="b1", name="b1")
            w2_sb = cpool.tile([36, 32], F16, tag="w2", name="w2")
            b2_sb = cpool.tile([32, 1], F32, tag="b2", name="b2")
            wsc_sb = cpool.tile([32, 1], F16, tag="wsc", name="wsc")
            bsc_sb = cpool.tile([1, 1], F32, tag="bsc", name="bsc")
            ident = cpool.tile([128, 128], F16, tag="ident", name="ident")
            sci_sb = cpool.tile([128, NWORD // 128], I32, tag="sci", name="sci")

            # xh: slot 0 = per-step one-hot (DMA), slot 1 = h state (fp8)
            xh = spool.tile([128, 2, NWORD], F8, tag="xh", name="xh")
            c_sb = spool.tile([128, NWORD], F16, tag="c", name="c")
            # sifo slots: 0=sig(i) 1=sig(f) 2=sig(o) 3=sig(2g), double
            # buffered by step parity so step t+1's sigmoid never waits for
            # DVE's reads of step t's values (write-after-read)
            sifo = spool.tile([128, 8 * NWORD], F16, tag="sifo", name="sifo")
            sg = spool.tile([128, NWORD], F16, tag="sg", name="sg")
            sc = spool.tile([128, NWORD], F16, tag="sc", name="sc")

            # startup: the critical first-matmul inputs (lt + step-0 one-hot)
            # issue first, spread over engine queues that are idle at t=0 so
            # the ~900ns-per-issue DMA costs overlap. Everything else follows
            # on sync/gpsimd.
            nc.sync.dma_start(lt_sb[:], lt_in[:])
            w0 = Wx[0]
            nb0 = (w0 + BLK - 1) // BLK
            eng0 = [nc.scalar, nc.gpsimd, nc.sync, nc.scalar,
                    nc.gpsimd, nc.sync, nc.scalar, nc.gpsimd]
            for k in range(nb0):
                a, b = k * BLK, min(w0, (k + 1) * BLK)
                eng0[k].dma_start(xh[:, 0, a:b], oh_in[0, :, a:b])
            consts = [(w1_sb, w1_in), (b1_sb, b1_in), (w2_sb, w2_in),
                      (b2_sb, b2_in), (wsc_sb, wsc_in), (bsc_sb, bsc_in),
                      (sci_sb, sci_in)]
            if RES:
                consts.append((ltr_sb, ltr_in))
            for sb, dr in consts:
                nc.gpsimd.dma_start(sb[:], dr[:])

            # ------------------------------------------------ LSTM main loop
            with (
                tc.tile_pool(name="g4", bufs=2, space="PSUM") as g4pool,
                tc.tile_pool(name="cn", bufs=4) as cnpool,
            ):
                # gate column offsets inside a [128, 2048] 4-bank group
                CI, CF, CO, CG = 0, 512, 1024, 1536
                GS = {0: slice(0, H), 1: slice(H, 2 * H),
                      2: slice(2 * H, 3 * H), 3: slice(3 * H, 4 * H)}
                oready = []  # [(base, wkn)] tanh(c)+h pending

                def flush_list(lst):
                    if not lst:
                        return
                    base0 = lst[0][0]
                    span = lst[-1][0] + lst[-1][1] - base0
                    ps = lst[0][2]
                    lst.clear()
                    nc.scalar.activation(sc[:, base0:base0 + span],
                                         c_sb[:, base0:base0 + span], AF.Tanh)
                    nc.vector.tensor_tensor(
                        xh[:, 1, base0:base0 + span],
                        sifo[:, ps + 2 * NWORD + base0:
                             ps + 2 * NWORD + base0 + span],
                        sc[:, base0:base0 + span], ALU.mult)

                def finalize(host_grp, dbase, c4lo, c4hi):
                    # frozen columns are final: transpose [H,word]->[word,H]
                    # via the PE into the hosting group's g-bank (free once
                    # its sigmoid has read it), one merged PSUM->SBUF copy,
                    # then scatter rows in original word order.
                    gf16 = host_grp[:].bitcast(F16)
                    for c4 in range(c4lo, c4hi):
                        col = dbase + c4 * 128
                        po = 2 * CG + c4 * 128
                        nc.tensor.matmul(
                            gf16[:, po:po + 128], c_sb[:, col:col + 128],
                            ident[:], is_transpose=True,
                            skip_group_check=True)
                    cn = cnpool.tile([128, 512], F16, tag="cn", name="cn")
                    nc.vector.tensor_copy(
                        cn[:, c4lo * 128:c4hi * 128],
                        gf16[:, 2 * CG + c4lo * 128:2 * CG + c4hi * 128])
                    for c4 in range(c4lo, c4hi):
                        cc = (dbase + c4 * 128) // 128
                        nc.gpsimd.indirect_dma_start(
                            out=c_dram[:],
                            out_offset=bass.IndirectOffsetOnAxis(
                                ap=sci_sb[:, cc:cc + 1], axis=0),
                            in_=cn[:, c4 * 128:(c4 + 1) * 128],
                            in_offset=None,
                        )

                pd0 = min(4, (Wx[L - 1] + 127) // 128)

                # chunks of block 0 past W[L-1] freeze by step L-2 in
                # every core and can be scattered one step early
                make_identity(nc, ident[:])
                for t in range(L):
                    w = Wx[t]
                    wn = Wx[t + 1]
                    if w == 0:
                        continue
                    nb = (w + BLK - 1) // BLK
                    # leftover tanh(c)+h from the previous step: defer past
                    # this step's first block unless its columns overlap
                    carry = list(oready)
                    oready.clear()
                    if carry and carry[0][0] < BLK:
                        flush_list(carry)
                    if t > 0:  # t=0 one-hot is prefetched before the loop
                        for k in range(nb):
                            a, b = k * BLK, min(w, (k + 1) * BLK)
                            nc.sync.dma_start(xh[:, 0, a:b], oh_in[t, :, a:b])
                    ps = (t % 2) * 4 * NWORD
                    for k in range(nb):
                        base = k * BLK
                        wk = min(BLK, w - base)
                        wkn = max(0, min(BLK, wn - base))
                        grp = g4pool.tile([128, 4 * BLK], F32, tag="g4", name="g4")
                        # gate matmuls; i,f,o first so sigmoid starts early
                        # (no f at t=0 since c0=0; no o for dying blocks).
                        # The g matmul lands in a bank adjacent to the live
                        # gates so one strided sigmoid covers everything.
                        gates = [(0, CI, wk)]
                        if t > 0:
                            gates.append((1, CF, wk))
                        if wkn > 0:
                            gates.append((3, CO, wkn))
                            gcol, gslot = CG, 3
                        elif t > 0:
                            gcol, gslot = CO, 2
                        else:
                            gcol, gslot = CF, 1
                        gates.append((2, gcol, wk))
                        for m, col, gw in gates:
                            if t == 0:
                                # h is uninitialized: one-hot k-tile only
                                nc.tensor.matmul(
                                    grp[:, col:col + gw],
                                    lhsT=lt_sb[:, 0, GS[m]],
                                    rhs=xh[:, 0, base:base + gw],
                                    start=True, stop=not RES)
                                if RES:
                                    nc.tensor.matmul(
                                        grp[:, col:col + gw],
                                        lhsT=ltr_sb[:, 0, GS[m]],
                                        rhs=xh[:, 0, base:base + gw],
                                        start=False, stop=True)
                            else:
                                nc.tensor.matmul(
                                    grp[:, col:col + gw],
                                    lhsT=lt_sb[:, :, GS[m]],
                                    rhs=xh[:, :, base:base + gw],
                                    start=True, stop=not RES, perf_mode=DR)
                                if RES:
                                    nc.tensor.matmul(
                                        grp[:, col:col + gw],
                                        lhsT=ltr_sb[:, :, GS[m]],
                                        rhs=xh[:, :, base:base + gw],
                                        start=False, stop=True, perf_mode=DR)
                        # one strided sigmoid covers every live gate bank
                        if t == 0 and wkn > 0:      # i@0; o@1024,g@1536
                            nc.scalar.activation(sifo[:, ps + base:
                                                      ps + base + wk],
                                                 grp[:, 0:wk], AF.Sigmoid)
                            nc.scalar.activation(
                                view(sifo[:], ps + 2 * NWORD + base,
                                     [[NWORD, 2], [1, wk]]),
                                view(grp[:], CO, [[512, 2], [1, wk]]),
                                AF.Sigmoid)
                        elif t == 0:                # i@0, g@512
                            nc.scalar.activation(
                                view(sifo[:], ps + base, [[NWORD, 2], [1, wk]]),
                                view(grp[:], 0, [[512, 2], [1, wk]]),
                                AF.Sigmoid)
                        elif wkn > 0:               # i,f,o,g @ stride 512
                            nc.scalar.activation(
                                view(sifo[:], ps + base, [[NWORD, 4], [1, wk]]),
                                view(grp[:], 0, [[512, 4], [1, wk]]),
                                AF.Sigmoid)
                        else:                       # i,f,g @ stride 512
                            nc.scalar.activation(
                                view(sifo[:], ps + base, [[NWORD, 3], [1, wk]]),
                                view(grp[:], 0, [[512, 3], [1, wk]]),
                                AF.Sigmoid)
                        # DVE: G = 2*sig(2g)-1 = tanh(g), then c = f*c + i*G
                        gs = sifo[:, ps + gslot * NWORD + base:
                                  ps + gslot * NWORD + base + wk]
                        nc.vector.tensor_scalar(gs, gs, 2.0, 1.0,
                                                ALU.mult, ALU.subtract)
                        if t == 0:
                            nc.vector.tensor_tensor(
                                c_sb[:, base:base + wk],
                                sifo[:, ps + base:ps + base + wk],
                                gs, ALU.mult)
                        else:
                            nc.vector.tensor_tensor(
                                sg[:, base:base + wk],
                                sifo[:, ps + base:ps + base + wk],
                                gs, ALU.mult)
                            nc.vector.tensor_tensor(
                                c_sb[:, base:base + wk],
                                sifo[:, ps + NWORD + base:
                                     ps + NWORD + base + wk],
                                c_sb[:, base:base + wk], ALU.mult)
                            nc.vector.tensor_tensor(
                                c_sb[:, base:base + wk], sg[:, base:base + wk],
                                c_sb[:, base:base + wk], ALU.add)
                        if wkn > 0:
                            oready.append((base, wkn, ps))
                            # narrow steps: flush immediately so the next
                            # step's h dependency clears while the sigmoid
                            # of the later block still runs
                            if len(oready) == 2 or nb <= 2:
                                flush_list(oready)
                        else:
                            # dying blocks park g in the o-bank, so their
                            # own g-bank is a dependency-free transpose
                            # target; scatters start immediately. Block 0's
                            # tail chunks were pre-scattered at step L-2.
                            hi = pd0 if (base == 0 and t == L - 1) else 4
                            finalize(grp, base, 0, hi)
                        if k == 0:
                            flush_list(carry)
                        if t == L - 2 and k == nb - 1 and pd0 < 4:
                            finalize(grp, 0, pd0, 4)
                flush_list(oready)
            # ------------------------------------------------------- tail
            with (
                tc.tile_pool(name="tpsum", bufs=2, space="PSUM") as tpsum,
                tc.tile_pool(name="cpsum", bufs=2, space="PSUM") as cpsum,
                tc.tile_pool(name="small", bufs=1) as small,
            ):
                A = small.tile([128, NWORD], F16, tag="A", name="A")
                # A[p, (i*NEC+ec)*128 + h] = c_dram[ec*512 + p*4 + i, h]
                # split by word-index i into 4 parallel DMAs
                geng = [nc.sync, nc.scalar, nc.gpsimd, nc.sync]
                for i4 in range(NW):
                    srcap = bass.AP(
                        c_dram[:].tensor, i4 * H,
                        [[NW * H, 128], [BLK * H, NEC], [1, H]])
                    dst = bass.AP(
                        A.tensor, A.offset + i4 * NEC * 128,
                        [A.ap[0], [128, NEC], [1, H]])
                    geng[i4].dma_start(dst, srcap)

                WSEG = NEC * 128
                prod = small.tile([128, NWORD], F16, tag="prod", name="prod")
                D0 = small.tile([128, NW * NEC], F32, tag="D0", name="D0")
                S = small.tile([128, NW * NEC], F32, tag="S", name="S")
                C6 = small.tile([128, 6 * NEC], F32, tag="C6", name="C6")
                C6h = small.tile([128, 6 * NEC], F16, tag="C6h", name="C6h")
                SS = small.tile([128, 6 * NEC], F32, tag="SS", name="SS")
                cos6 = small.tile([6, PER], F16, tag="cos6", name="cos6")

                # self-dots first (DVE) so the rsqrt chain overlaps the
                # pair products
                for i in range(NW):
                    nc.vector.tensor_tensor(
                        prod[:, i * WSEG:(i + 1) * WSEG],
                        A[:, i * WSEG:(i + 1) * WSEG],
                        A[:, i * WSEG:(i + 1) * WSEG], ALU.mult)
                nc.vector.tensor_reduce(
                    D0[:],
                    prod[:].rearrange("p (i e h) -> p (i e) h", i=NW, e=NEC),
                    axis=mybir.AxisListType.X, op=ALU.add)
                nc.vector.tensor_scalar_max(D0[:], D0[:], 1e-30)
                nc.scalar.activation(S[:], D0[:], AF.Ln)
                nc.scalar.activation(S[:], S[:], AF.Exp, scale=-0.5)
                for kp, (i, j) in enumerate(P6):
                    nc.vector.tensor_tensor(
                        prod[:, :WSEG], A[:, i * WSEG:(i + 1) * WSEG],
                        A[:, j * WSEG:(j + 1) * WSEG], ALU.mult)
                    nc.vector.tensor_reduce(
                        C6[:, kp * NEC:(kp + 1) * NEC],
                        prod[:, :WSEG].rearrange("p (e h) -> p e h", e=NEC),
                        axis=mybir.AxisListType.X, op=ALU.add)
                for kp, (i, j) in enumerate(P6):
                    nc.vector.tensor_tensor(
                        SS[:, kp * NEC:(kp + 1) * NEC],
                        S[:, i * NEC:(i + 1) * NEC],
                        S[:, j * NEC:(j + 1) * NEC], ALU.mult)
                nc.vector.tensor_tensor(C6h[:], C6[:], SS[:], ALU.mult)
                r1 = small.tile([36, PER], F16, tag="r1", name="r1")
                r2 = small.tile([32, PER], F16, tag="r2", name="r2")
                o_sb = small.tile([1, PER], F32, tag="o", name="o")

                def transp(ecs):
                    for ec in ecs:
                        pt_ = tpsum.tile([128, 128], F16, tag="tc", name="tc")
                        cview = bass.AP(C6h.tensor, C6h.offset + ec,
                                        [C6h.ap[0], [NEC, 6]])
                        nc.tensor.transpose(pt_[:6, :], cview, ident[:])
                        nc.vector.tensor_copy(
                            cos6[:, ec * 128:(ec + 1) * 128], pt_[:6, :])

                # two example-halves pipelined through the CNN stages so
                # each ACT stage overlaps the other half's matmuls
                sl = [slice(0, 512), slice(512, 1024)]
                transp(range(0, NEC // 2))
                p1 = [None, None]
                p2 = [None, None]
                p3 = [None, None]
                p1[0] = cpsum.tile([36, 512], F32, tag="cp1", name="cp1")
                nc.tensor.matmul(p1[0][:], lhsT=w1_sb[:], rhs=cos6[:, sl[0]],
                                 start=True, stop=True)
                transp(range(NEC // 2, NEC))
                p1[1] = cpsum.tile([36, 512], F32, tag="cp1", name="cp1")
                nc.tensor.matmul(p1[1][:], lhsT=w1_sb[:], rhs=cos6[:, sl[1]],
                                 start=True, stop=True)
                for h in range(2):
                    nc.scalar.activation(r1[:, sl[h]], p1[h][:], AF.Relu,
                                         bias=b1_sb[:, 0:1])
                    p2[h] = cpsum.tile([32, 512], F32, tag="cp1", name="cp1")
                    nc.tensor.matmul(p2[h][:], lhsT=w2_sb[:], rhs=r1[:, sl[h]],
                                     start=True, stop=True)
                for h in range(2):
                    nc.scalar.activation(r2[:, sl[h]], p2[h][:], AF.Relu,
                                         bias=b2_sb[:, 0:1])
                    p3[h] = cpsum.tile([1, 512], F32, tag="cp1", name="cp1")
                    nc.tensor.matmul(p3[h][:], lhsT=wsc_sb[:], rhs=r2[:, sl[h]],
                                     start=True, stop=True)
                for h in range(2):
                    nc.scalar.activation(o_sb[:, sl[h]], p3[h][:], AF.Sigmoid,
                                         bias=bsc_sb[0:1, 0:1])
                nc.sync.dma_start(out_d[:], o_sb[:])

    return nc


_prog_cache = {}


def _get_program(W):
    key = tuple(int(x) for x in W)
    if key not in _prog_cache:
        _prog_cache[key] = _build_program(key)
    return _prog_cache[key]


def _run(inputs, trace=False):
    consts = _build_consts(inputs)
    word_ids = np.asarray(inputs["word_ids"])
    lengths = np.asarray(inputs["lengths"])

    preps = []
    for c in range(NCORES):
        sl = slice(c * PER, (c + 1) * PER)
        preps.append(_core_prep(word_ids[sl], lengths[sl]))
    Nt_max = np.stack([p[2] for p in preps]).max(0)
    W = tuple(int(min(NWORD, -(-int(n) // 16) * 16)) for n in Nt_max)

    lt_f8 = consts["LT"].astype(F8NP)
    ltr_f8 = consts["LTR"].astype(F8NP)
    in_maps = []
    for c in range(NCORES):
        wid_s, lens_s, _, perm = preps[c]
        in_maps.append({
            "oh": _build_onehot(wid_s, lens_s, W),
            "scidx": _build_scidx(perm),
            "lt": lt_f8, "ltr": ltr_f8,
            "w1": consts["W1eff"].astype(np.float16),
            "b1": consts["b1eff"],
            "w2": consts["W2eff"].astype(np.float16),
            "b2": consts["b2eff"],
            "wsc": consts["Wsc"].astype(np.float16),
            "bsc": np.full((1, 1), consts["bsc"], np.float32),
        })

    nc = _get_program(W)
    _spill_excess_waits(nc)  # idempotent; HW-compile only
    res = run_bass_kernel_spmd(nc, in_maps, list(range(NCORES)), trace=trace)
    out = np.concatenate([np.asarray(r["out"]).reshape(PER) for r in res.results])
    return out.reshape(B, 1).astype(np.float32), res.exec_time_ns


def kernel(**inputs):
    return _run(inputs)[0]


# revision 49
# speedup vs baseline: 1.0313x; 1.0148x over previous
"""Trainium2 Bass kernel for the char-LSTM word-similarity CNN scorer.

Problem: B=8192 examples x NW=4 words x L=16 chars. Per word: char
embeddings -> masked LSTMCell over <=16 steps -> cell state c [128].
Per example: 4x4 cosine matrix of the word reps -> 2-layer 2x2-valid
CNN -> linear scorer -> sigmoid.

Strategy (pure data parallel, 1024 examples / 4096 words per core):
 - Host folds emb @ W_ih.T + (b_ih + b_hh) into a [128, 512] table
   (row 64 = "freeze" flag driving f->1, i->0 for words past their
   length); per-step char inputs are a one-hot matrix.
 - Words sorted by length (desc) on host; step t processes exactly the
   active width N_t (rounded to 16), in 512-column PSUM blocks.
 - fp8e4 DoubleRow matmuls: each gate's pre-activation is ONE PE
   matmul with two k-tiles -- (one-hot @ G65) + (h @ WhhT) -- at 0.5
   cycles per output column (plus an optional residual-table pass that
   cancels the fp8 weight quantization error). The one-hot operand is
   exact in fp8; h is stored as fp8e4 in the same SBUF tile as the
   one-hot so both k-tiles come from a single strided access pattern.
 - ACT work minimized: ALL FOUR gate activations run as ONE strided-AP
   sigmoid per block (g-gate table rows are pre-doubled so tanh(g) =
   2*sigmoid(2g)-1; the fixup is a cheap DVE tensor_scalar);
   o-gate/tanh(c)/h only computed for words still alive at the next
   step; no f-gate at t=0 (c0=0); no memsets (c=i*g at t=0, h never
   read at t=0). The sigmoid output buffer is double-buffered by step
   parity so it never write-after-read stalls against DVE.
 - PSUM: 4-bank gate groups (i|f|o|g), double buffered. Dying blocks
   park g in the o-bank so their own g-bank is a dependency-free PE
   transpose target.
 - As each 512-word block freezes, its c columns are transposed
   ([H,word]->[word,H]) and indirect-DMA scattered to DRAM in original
   word order -- overlapped with the remaining LSTM steps. Chunks of
   the last block past W[15] are provably frozen a step early and
   pre-scattered. The tail is then: one contiguous gather, all 10 dot
   products (6 pairs + 4 self-dots) on DVE, rsqrt norm scaling, and
   the tiny CNN/scorer lowered to matmuls pipelined in two
   example-halves.
"""

import os
import sys

for _p in ("/opt/trn_rl_repo",):
    if _p not in sys.path and os.path.isdir(_p):
        sys.path.insert(0, _p)

import numpy as np
import ml_dtypes

import concourse.bass as bass
import concourse.mybir as mybir
import concourse.tile as tile
from concourse.bass_utils import run_bass_kernel_spmd
from concourse.masks import make_identity

F8NP = ml_dtypes.float8_e4m3

# This container's walrus build rejects CTRL instructions (Drain) carrying
# more than 2 sync waits ("Too many sync wait commands" in setupSyncWait).
# Tile's kernel-tail drain accumulates one wait per engine/DMA-queue sem, so
# redistribute: keep one wait on the drain, move the rest onto nofuse NOPs
# that execute before the all-engine barrier. Semantics are unchanged (all
# waits still complete before the barrier / semaphore teardown).
def _patched_drain_and_barrier(self, tick_clock, wait_clock):
    nc = self.nc
    drain_inst = nc.sync.drain()
    wait_clock.add_sem_waits(
        drain_inst.ins, tile.ScopedClock({None: tick_clock.global_clock})
    )
    waits = list(drain_inst.ins.sync_info.on_wait)
    if len(waits) > 1:
        drain_inst.ins.sync_info.on_wait = waits[:1]
        for k in range(1, len(waits)):
            nop = nc.sync.nop(nofuse=True, hint="drain_wait_spill")
            if nop.ins.sync_info is None:
                nop.ins.sync_info = mybir.SyncInfo(on_wait=[], on_update=[])
            nop.ins.sync_info.on_wait = [waits[k]]
    nc.all_engine_barrier()
    assert self.sems is not None
    popped = nc._tile_sem_poison_stack.pop()
    assert popped is self._sem_poison
    nc.clear_and_free_semaphores(list(self.sems.allocated().values()))
    nc.all_engine_barrier()


tile.TileContext._drain_and_barrier = _patched_drain_and_barrier

def _spill_excess_waits(nc):
    """Walrus here rejects instructions with more than ~2 sync waits. Spill
    excess waits onto same-engine NoOps inserted just before the instruction
    (engines dispatch in program order, so waiting earlier on the same engine
    is equivalent)."""
    cnt = [0]
    for fn in nc.m.functions:
        for bb in fn.blocks:
            insts = list(bb.instructions)
            out = []
            changed = False
            for inst in insts:
                si = inst.sync_info
                waits = list(si.on_wait) if si is not None and si.on_wait else []
                max_waits = 1
                if len(waits) > max_waits:
                    changed = True
                    keep = waits[-max_waits:]
                    extra = waits[:-max_waits]
                    for j in range(0, len(extra), max_waits):
                        cnt[0] += 1
                        nop = mybir.InstNoOp(name=f"I-spillw-{cnt[0]}", ins=[], outs=[])
                        nop.engine = inst.engine
                        nop.sync_info = mybir.SyncInfo(
                            on_wait=extra[j:j + max_waits], on_update=[])
                        nop.bass_nofuse = True
                        nop.bass_priority = 0
                        nop.text_hint = "spillw"
                        nop.debug = inst.debug
                        out.append(nop)
                    si.on_wait = keep
                out.append(inst)
            if changed:
                bb.instructions = out

B, NW, L, E, H, V = 8192, 4, 16, 128, 128, 64
NCORES = 8
PER = B // NCORES          # 1024 examples per core
NWORD = PER * NW           # 4096 words per core
NEC = PER // 128           # 8 example-chunks of 128
BLK = 512
NBLK = NWORD // BLK
FB = 30.0                  # freeze bias magnitude
RES = False                # second DoubleRow pass with fp8 residual tables
F32 = mybir.dt.float32
F16 = mybir.dt.float16
F8 = mybir.dt.float8e4
I32 = mybir.dt.int32
AF = mybir.ActivationFunctionType
ALU = mybir.AluOpType
DR = mybir.MatmulPerfMode.DoubleRow

P6 = [(0, 1), (0, 2), (0, 3), (1, 2), (1, 3), (2, 3)]


# ----------------------------------------------------------------- host prep

def _f8rt(x):
    return x.astype(F8NP).astype(np.float32)


def _build_consts(inp):
    emb = np.asarray(inp["emb_i"], np.float32)
    W_ih = np.asarray(inp["W_ih"], np.float32)
    W_hh = np.asarray(inp["W_hh"], np.float32)
    b = np.asarray(inp["b_ih"], np.float32) + np.asarray(inp["b_hh"], np.float32)
    G65 = np.zeros((128, 4 * H), np.float32)
    G65[:V] = emb @ W_ih.T + b
    G65[V, 0:H] = -FB
    G65[V, H:2 * H] = +FB
    WhhT = np.ascontiguousarray(W_hh.T)  # [H, 4H]
    # g-gate pre-activations are doubled so tanh(g) can be evaluated on the
    # sigmoid table together with i,f,o in one strided ACT instruction:
    # tanh(x) = 2*sigmoid(2x) - 1 (the 2s-1 fixup runs on DVE)
    G65[:, 2 * H:3 * H] *= 2.0
    WhhT[:, 2 * H:3 * H] *= 2.0

    # interleaved DoubleRow lhsT tables [128, 2, 4H]: k-tile 0 = G65
    # (one-hot side), k-tile 1 = WhhT (h side); plus fp8 residuals.
    LT = np.zeros((128, 2, 4 * H), np.float32)
    LTR = np.zeros((128, 2, 4 * H), np.float32)
    LT[:, 0, :] = _f8rt(G65)
    LT[:, 1, :] = _f8rt(WhhT)
    LTR[:, 0, :] = _f8rt(G65 - LT[:, 0, :])
    LTR[:, 1, :] = _f8rt(WhhT - LT[:, 1, :])

    w1 = np.asarray(inp["conv1_w"], np.float32)
    b1 = np.asarray(inp["conv1_b"], np.float32)
    w2 = np.asarray(inp["conv2_w"], np.float32)
    b2 = np.asarray(inp["conv2_b"], np.float32)
    ws = np.asarray(inp["scorer_w"], np.float32)
    bs = float(np.asarray(inp["scorer_b"], np.float32)[0])

    p6idx = {p: i for i, p in enumerate(P6)}
    W1eff = np.zeros((6, 36), np.float32)
    b1eff = np.zeros((36, 1), np.float32)
    for c in range(4):
        for y in range(3):
            for x in range(3):
                m = c * 9 + y * 3 + x
                b1eff[m, 0] += b1[c]
                for dy in range(2):
                    for dx in range(2):
                        a, bb = y + dy, x + dx
                        w = w1[c, 0, dy, dx]
                        if a == bb:
                            b1eff[m, 0] += w
                        else:
                            W1eff[p6idx[(min(a, bb), max(a, bb))], m] += w
    W2eff = np.zeros((36, 32), np.float32)
    b2eff = np.zeros((32, 1), np.float32)
    for c2 in range(8):
        for y in range(2):
            for x in range(2):
                m = c2 * 4 + y * 2 + x
                b2eff[m, 0] = b2[c2]
                for c1 in range(4):
                    for dy in range(2):
                        for dx in range(2):
                            W2eff[c1 * 9 + (y + dy) * 3 + (x + dx), m] += w2[c2, c1, dy, dx]
    Wsc = ws[0].astype(np.float32).reshape(32, 1)
    return dict(LT=LT, LTR=LTR, W1eff=W1eff, b1eff=b1eff,
                W2eff=W2eff, b2eff=b2eff, Wsc=Wsc, bsc=bs)


def _core_prep(word_ids_c, lengths_c):
    wid = np.asarray(word_ids_c).reshape(NWORD, L)
    lens = np.asarray(lengths_c).reshape(NWORD)
    perm = np.argsort(-lens, kind="stable").astype(np.int32)
    wid_s = wid[perm]
    lens_s = lens[perm]
    Nt = (np.arange(L)[:, None] < lens_s[None, :]).sum(1)  # [L]
    return wid_s, lens_s, Nt, perm


def _build_onehot(wid_s, lens_s, widths):
    oh = np.zeros((L, 128, NWORD), F8NP)
    one = np.ones((), F8NP)
    cols = np.arange(NWORD)
    for t in range(L):
        n = int(widths[t])
        if n == 0:
            continue
        alive = lens_s[:n] > t
        oh[t, wid_s[:n, t], cols[:n]] = np.where(alive, one, np.zeros((), F8NP))
        oh[t, V, cols[:n]] = np.where(alive, np.zeros((), F8NP), one)
    return oh


def _build_scidx(perm):
    # scidx[p, cc] = original index of sorted word cc*128+p
    return np.ascontiguousarray(
        perm.reshape(NWORD // 128, 128).T).astype(np.int32)


# -------------------------------------------------------------- bass program

def _build_program(W):
    """W: tuple of per-step active widths (len 16, multiples of 16, desc)."""
    nc = bass.Bass()
    Wx = list(W) + [0]

    oh_in = nc.dram_tensor("oh", [L, 128, NWORD], F8, kind="ExternalInput")
    sci_in = nc.dram_tensor("scidx", [128, NWORD // 128], I32, kind="ExternalInput")
    lt_in = nc.dram_tensor("lt", [128, 2, 4 * H], F8, kind="ExternalInput")
    ltr_in = nc.dram_tensor("ltr", [128, 2, 4 * H], F8, kind="ExternalInput")
    w1_in = nc.dram_tensor("w1", [6, 36], F16, kind="ExternalInput")
    b1_in = nc.dram_tensor("b1", [36, 1], F32, kind="ExternalInput")
    w2_in = nc.dram_tensor("w2", [36, 32], F16, kind="ExternalInput")
    b2_in = nc.dram_tensor("b2", [32, 1], F32, kind="ExternalInput")
    wsc_in = nc.dram_tensor("wsc", [32, 1], F16, kind="ExternalInput")
    bsc_in = nc.dram_tensor("bsc", [1, 1], F32, kind="ExternalInput")
    out_d = nc.dram_tensor("out", [1, PER], F32, kind="ExternalOutput")
    c_dram = nc.dram_tensor("cscratch", [NWORD, H], F16)

    def view(ap, off, dims):
        return bass.AP(ap.tensor, ap.offset + off, [ap.ap[0]] + dims)

    with tile.TileContext(nc) as tc:
        with (
            tc.tile_pool(name="const", bufs=1) as cpool,
            tc.tile_pool(name="state", bufs=1) as spool,
        ):
            lt_sb = cpool.tile([128, 2, 4 * H], F8, tag="lt", name="lt")
            ltr_sb = cpool.tile([128, 2, 4 * H], F8, tag="ltr", name="ltr") \
                if RES else None
            w1_sb = cpool.tile([6, 36], F16, tag="w1", name="w1")
            b1_sb = cpool.tile([36, 1], F32, tag="b1", name="b1")
            w2_sb = cpool.tile([36, 32], F16, tag="w2", name="w2")
            b2_sb = cpool.tile([32, 1], F32, tag="b2", name="b2")
            wsc_sb = cpool.tile([32, 1], F16, tag="wsc", name="wsc")
            bsc_sb = cpool.tile([1, 1], F32, tag="bsc", name="bsc")
            ident = cpool.tile([128, 128], F16, tag="ident", name="ident")
            sci_sb = cpool.tile([128, NWORD // 128], I32, tag="sci", name="sci")

            # xh: slot 0 = per-step one-hot (DMA), slot 1 = h state (fp8)
            xh = spool.tile([128, 2, NWORD], F8, tag="xh", name="xh")
            c_sb = spool.tile([128, NWORD], F16, tag="c", name="c")
            # sifo slots: 0=sig(i) 1=sig(f) 2=sig(o) 3=sig(2g), double
            # buffered by step parity so step t+1's sigmoid never waits for
            # DVE's reads of step t's values (write-after-read)
            sifo = spool.tile([128, 8 * NWORD], F16, tag="sifo", name="sifo")
            sg = spool.tile([128, NWORD], F16, tag="sg", name="sg")
            sc = spool.tile([128, NWORD], F16, tag="sc", name="sc")

            # startup: the critical first-matmul inputs (lt + step-0 one-hot)
            # issue first, spread over engine queues that are idle at t=0 so
            # the ~900ns-per-issue DMA costs overlap. Everything else follows
            # on sync/gpsimd.
            nc.sync.dma_start(lt_sb[:], lt_in[:])
            w0 = Wx[0]
            nb0 = (w0 + BLK - 1) // BLK
            eng0 = [nc.scalar, nc.gpsimd, nc.sync, nc.scalar,
                    nc.gpsimd, nc.sync, nc.scalar, nc.gpsimd]
            for k in range(nb0):
                a, b = k * BLK, min(w0, (k + 1) * BLK)
                eng0[k].dma_start(xh[:, 0, a:b], oh_in[0, :, a:b])
            consts = [(w1_sb, w1_in), (b1_sb, b1_in), (w2_sb, w2_in),
                      (b2_sb, b2_in), (wsc_sb, wsc_in), (bsc_sb, bsc_in),
                      (sci_sb, sci_in)]
            if RES:
                consts.append((ltr_sb, ltr_in))
            for sb, dr in consts:
                nc.gpsimd.dma_start(sb[:], dr[:])

            # ------------------------------------------------ LSTM main loop
            with (
                tc.tile_pool(name="g4", bufs=2, space="PSUM") as g4pool,
                tc.tile_pool(name="cn", bufs=4) as cnpool,
            ):
                # gate column offsets inside a [128, 2048] 4-bank group
                CI, CF, CO, CG = 0, 512, 1024, 1536
                GS = {0: slice(0, H), 1: slice(H, 2 * H),
                      2: slice(2 * H, 3 * H), 3: slice(3 * H, 4 * H)}
                oready = []  # [(base, wkn)] tanh(c)+h pending

                def flush_list(lst):
                    if not lst:
                        return
                    base0 = lst[0][0]
                    span = lst[-1][0] + lst[-1][1] - base0
                    ps = lst[0][2]
                    lst.clear()
                    nc.scalar.activation(sc[:, base0:base0 + span],
                                         c_sb[:, base0:base0 + span], AF.Tanh)
                    nc.vector.tensor_tensor(
                        xh[:, 1, base0:base0 + span],
                        sifo[:, ps + 2 * NWORD + base0:
                             ps + 2 * NWORD + base0 + span],
                        sc[:, base0:base0 + span], ALU.mult)

                def finalize(host_grp, dbase, c4lo, c4hi):
                    # frozen columns are final: transpose [H,word]->[word,H]
                    # via the PE into the hosting group's g-bank (free once
                    # its sigmoid has read it), one merged PSUM->SBUF copy,
                    # then scatter rows in original word order.
                    gf16 = host_grp[:].bitcast(F16)
                    for c4 in range(c4lo, c4hi):
                        col = dbase + c4 * 128
                        po = 2 * CG + c4 * 128
                        nc.tensor.matmul(
                            gf16[:, po:po + 128], c_sb[:, col:col + 128],
                            ident[:], is_transpose=True,
                            skip_group_check=True)
                    cn = cnpool.tile([128, 512], F16, tag="cn", name="cn")
                    nc.vector.tensor_copy(
                        cn[:, c4lo * 128:c4hi * 128],
                        gf16[:, 2 * CG + c4lo * 128:2 * CG + c4hi * 128])
                    for c4 in range(c4lo, c4hi):
                        cc = (dbase + c4 * 128) // 128
                        nc.gpsimd.indirect_dma_start(
                            out=c_dram[:],
                            out_offset=bass.IndirectOffsetOnAxis(
                                ap=sci_sb[:, cc:cc + 1], axis=0),
                            in_=cn[:, c4 * 128:(c4 + 1) * 128],
                            in_offset=None,
                        )

                pd0 = min(4, (Wx[L - 1] + 127) // 128)

                # chunks of block 0 past W[L-1] freeze by step L-2 in
                # every core and can be scattered one step early
                make_identity(nc, ident[:])
                for t in range(L):
                    w = Wx[t]
                    wn = Wx[t + 1]
                    if w == 0:
                        continue
                    nb = (w + BLK - 1) // BLK
                    # leftover tanh(c)+h from the previous step: defer past
                    # this step's first block unless its columns overlap
                    carry = list(oready)
                    oready.clear()
                    if carry and carry[0][0] < BLK:
                        flush_list(carry)
                    if t > 0:  # t=0 one-hot is prefetched before the loop
                        for k in range(nb):
                            a, b = k * BLK, min(w, (k + 1) * BLK)
                            nc.sync.dma_start(xh[:, 0, a:b], oh_in[t, :, a:b])
                    ps = (t % 2) * 4 * NWORD
                    for k in range(nb):
                        base = k * BLK
                        wk = min(BLK, w - base)
                        wkn = max(0, min(BLK, wn - base))
                        grp = g4pool.tile([128, 4 * BLK], F32, tag="g4", name="g4")
                        # gate matmuls; i,f,o first so sigmoid starts early
                        # (no f at t=0 since c0=0; no o for dying blocks).
                        # The g matmul lands in a bank adjacent to the live
                        # gates so one strided sigmoid covers everything.
                        gates = [(0, CI, wk)]
                        if t > 0:
                            gates.append((1, CF, wk))
                        if wkn > 0:
                            gates.append((3, CO, wkn))
                            gcol, gslot = CG, 3
                        elif t > 0:
                            gcol, gslot = CO, 2
                        else:
                            gcol, gslot = CF, 1
                        gates.append((2, gcol, wk))
                        for m, col, gw in gates:
                            if t == 0:
                                # h is uninitialized: one-hot k-tile only
                                nc.tensor.matmul(
                                    grp[:, col:col + gw],
                                    lhsT=lt_sb[:, 0, GS[m]],
                                    rhs=xh[:, 0, base:base + gw],
                                    start=True, stop=not RES)
                                if RES:
                                    nc.tensor.matmul(
                                        grp[:, col:col + gw],
                                        lhsT=ltr_sb[:, 0, GS[m]],
                                        rhs=xh[:, 0, base:base + gw],
                                        start=False, stop=True)
                            else:
                                nc.tensor.matmul(
                                    grp[:, col:col + gw],
                                    lhsT=lt_sb[:, :, GS[m]],
                                    rhs=xh[:, :, base:base + gw],
                                    start=True, stop=not RES, perf_mode=DR)
                                if RES:
                                    nc.tensor.matmul(
                                        grp[:, col:col + gw],
                                        lhsT=ltr_sb[:, :, GS[m]],
                                        rhs=xh[:, :, base:base + gw],
                                        start=False, stop=True, perf_mode=DR)
                        # one strided sigmoid covers every live gate bank
                        if t == 0 and wkn > 0:      # i@0; o@1024,g@1536
                            nc.scalar.activation(sifo[:, ps + base:
                                                      ps + base + wk],
                                                 grp[:, 0:wk], AF.Sigmoid)
                            nc.scalar.activation(
                                view(sifo[:], ps + 2 * NWORD + base,
                                     [[NWORD, 2], [1, wk]]),
                                view(grp[:], CO, [[512, 2], [1, wk]]),
                                AF.Sigmoid)
                        elif t == 0:                # i@0, g@512
                            nc.scalar.activation(
                                view(sifo[:], ps + base, [[NWORD, 2], [1, wk]]),
                                view(grp[:], 0, [[512, 2], [1, wk]]),
                                AF.Sigmoid)
                        elif wkn > 0:               # i,f,o,g @ stride 512
                            nc.scalar.activation(
                                view(sifo[:], ps + base, [[NWORD, 4], [1, wk]]),
                                view(grp[:], 0, [[512, 4], [1, wk]]),
                                AF.Sigmoid)
                        else:                       # i,f,g @ stride 512
                            nc.scalar.activation(
                                view(sifo[:], ps + base, [[NWORD, 3], [1, wk]]),
                                view(grp[:], 0, [[512, 3], [1, wk]]),
                                AF.Sigmoid)
                        # DVE: G = 2*sig(2g)-1 = tanh(g), then c = f*c + i*G
                        gs = sifo[:, ps + gslot * NWORD + base:
                                  ps + gslot * NWORD + base + wk]
                        nc.vector.tensor_scalar(gs, gs, 2.0, 1.0,
                                                ALU.mult, ALU.subtract)
                        if t == 0:
                            nc.vector.tensor_tensor(
                                c_sb[:, base:base + wk],
                                sifo[:, ps + base:ps + base + wk],
                                gs, ALU.mult)
                        else:
                            nc.vector.tensor_tensor(
                                sg[:, base:base + wk],
                                sifo[:, ps + base:ps + base + wk],
                                gs, ALU.mult)
                            nc.vector.tensor_tensor(
                                c_sb[:, base:base + wk],
                                sifo[:, ps + NWORD + base:
                                     ps + NWORD + base + wk],
                                c_sb[:, base:base + wk], ALU.mult)
                            nc.vector.tensor_tensor(
                                c_sb[:, base:base + wk], sg[:, base:base + wk],
                                c_sb[:, base:base + wk], ALU.add)
                        if wkn > 0:
                            oready.append((base, wkn, ps))
                            # narrow steps: flush immediately so the next
                            # step's h dependency clears while the sigmoid
                            # of the later block still runs
                            if len(oready) == 2 or nb <= 2:
                                flush_list(oready)
                        else:
                            # dying blocks park g in the o-bank, so their
                            # own g-bank is a dependency-free transpose
                            # target; scatters start immediately. Block 0's
                            # tail chunks were pre-scattered at step L-2.
                            hi = pd0 if (base == 0 and t == L - 1) else 4
                            finalize(grp, base, 0, hi)
                        if k == 0:
                            flush_list(carry)
                        if t == L - 2 and k == nb - 1 and pd0 < 4:
                            finalize(grp, 0, pd0, 4)
                flush_list(oready)
            # ------------------------------------------------------- tail
            with (
                tc.tile_pool(name="tpsum", bufs=2, space="PSUM") as tpsum,
                tc.tile_pool(name="cpsum", bufs=2, space="PSUM") as cpsum,
                tc.tile_pool(name="small", bufs=1) as small,
            ):
                A = small.tile([128, NWORD], F16, tag="A", name="A")
                # A[p, (i*NEC+ec)*128 + h] = c_dram[ec*512 + p*4 + i, h]
                # split by word-index i into 4 parallel DMAs
                geng = [nc.sync, nc.scalar, nc.gpsimd, nc.sync]
                for i4 in range(NW):
                    srcap = bass.AP(
                        c_dram[:].tensor, i4 * H,
                        [[NW * H, 128], [BLK * H, NEC], [1, H]])
                    dst = bass.AP(
                        A.tensor, A.offset + i4 * NEC * 128,
                        [A.ap[0], [128, NEC], [1, H]])
                    geng[i4].dma_start(dst, srcap)

                WSEG = NEC * 128
                prod = small.tile([128, NWORD], F16, tag="prod", name="prod")
                D0 = small.tile([128, NW * NEC], F32, tag="D0", name="D0")
                S = small.tile([128, NW * NEC], F32, tag="S", name="S")
                C6 = small.tile([128, 6 * NEC], F32, tag="C6", name="C6")
                C6h = small.tile([128, 6 * NEC], F16, tag="C6h", name="C6h")
                SS = small.tile([128, 6 * NEC], F32, tag="SS", name="SS")
                cos6 = small.tile([6, PER], F16, tag="cos6", name="cos6")

                # self-dots first: words 0-1 on the (tail-idle) ACT
                # engine via Square+accumulator, words 2-3 on DVE, so the
                # rsqrt chain clears while DVE moves on to pair products
                junk = small.tile([128, 128], F16, tag="junk", name="junk")
                for b in range(2 * NEC):
                    nc.scalar.activation(
                        junk[:], A[:, b * 128:(b + 1) * 128], AF.Square,
                        accum_out=D0[:, b:b + 1])
                for i in (2, 3):
                    nc.vector.tensor_tensor(
                        prod[:, (i - 2) * WSEG:(i - 1) * WSEG],
                        A[:, i * WSEG:(i + 1) * WSEG],
                        A[:, i * WSEG:(i + 1) * WSEG], ALU.mult)
                nc.vector.tensor_reduce(
                    D0[:, 2 * NEC:],
                    prod[:, :2 * WSEG].rearrange("p (i e h) -> p (i e) h",
                                                 i=2, e=NEC),
                    axis=mybir.AxisListType.X, op=ALU.add)
                nc.vector.tensor_scalar_max(D0[:], D0[:], 1e-30)
                nc.scalar.activation(S[:], D0[:], AF.Ln)
                nc.scalar.activation(S[:], S[:], AF.Exp, scale=-0.5)
                for kp, (i, j) in enumerate(P6):
                    nc.vector.tensor_tensor(
                        prod[:, :WSEG], A[:, i * WSEG:(i + 1) * WSEG],
                        A[:, j * WSEG:(j + 1) * WSEG], ALU.mult)
                    nc.vector.tensor_reduce(
                        C6[:, kp * NEC:(kp + 1) * NEC],
                        prod[:, :WSEG].rearrange("p (e h) -> p e h", e=NEC),
                        axis=mybir.AxisListType.X, op=ALU.add)
                for kp, (i, j) in enumerate(P6):
                    nc.vector.tensor_tensor(
                        SS[:, kp * NEC:(kp + 1) * NEC],
                        S[:, i * NEC:(i + 1) * NEC],
                        S[:, j * NEC:(j + 1) * NEC], ALU.mult)
                nc.vector.tensor_tensor(C6h[:], C6[:], SS[:], ALU.mult)
                r1 = small.tile([36, PER], F16, tag="r1", name="r1")
                r2 = small.tile([32, PER], F16, tag="r2", name="r2")
                o_sb = small.tile([1, PER], F32, tag="o", name="o")

                def transp(ecs):
                    for ec in ecs:
                        pt_ = tpsum.tile([128, 128], F16, tag="tc", name="tc")
                        cview = bass.AP(C6h.tensor, C6h.offset + ec,
                                        [C6h.ap[0], [NEC, 6]])
                        nc.tensor.transpose(pt_[:6, :], cview, ident[:])
                        nc.vector.tensor_copy(
                            cos6[:, ec * 128:(ec + 1) * 128], pt_[:6, :])

                # two example-halves pipelined through the CNN stages so
                # each ACT stage overlaps the other half's matmuls
                sl = [slice(0, 512), slice(512, 1024)]
                transp(range(0, NEC // 2))
                p1 = [None, None]
                p2 = [None, None]
                p3 = [None, None]
                p1[0] = cpsum.tile([36, 512], F32, tag="cp1", name="cp1")
                nc.tensor.matmul(p1[0][:], lhsT=w1_sb[:], rhs=cos6[:, sl[0]],
                                 start=True, stop=True)
                transp(range(NEC // 2, NEC))
                p1[1] = cpsum.tile([36, 512], F32, tag="cp1", name="cp1")
                nc.tensor.matmul(p1[1][:], lhsT=w1_sb[:], rhs=cos6[:, sl[1]],
                                 start=True, stop=True)
                for h in range(2):
                    nc.scalar.activation(r1[:, sl[h]], p1[h][:], AF.Relu,
                                         bias=b1_sb[:, 0:1])
                    p2[h] = cpsum.tile([32, 512], F32, tag="cp1", name="cp1")
                    nc.tensor.matmul(p2[h][:], lhsT=w2_sb[:], rhs=r1[:, sl[h]],
                                     start=True, stop=True)
                for h in range(2):
                    nc.scalar.activation(r2[:, sl[h]], p2[h][:], AF.Relu,
                                         bias=b2_sb[:, 0:1])
                    p3[h] = cpsum.tile([1, 512], F32, tag="cp1", name="cp1")
                    nc.tensor.matmul(p3[h][:], lhsT=wsc_sb[:], rhs=r2[:, sl[h]],
                                     start=True, stop=True)
                for h in range(2):
                    nc.scalar.activation(o_sb[:, sl[h]], p3[h][:], AF.Sigmoid,
                                         bias=bsc_sb[0:1, 0:1])
                nc.sync.dma_start(out_d[:], o_sb[:])

    return nc


_prog_cache = {}


def _get_program(W):
    key = tuple(int(x) for x in W)
    if key not in _prog_cache:
        _prog_cache[key] = _build_program(key)
    return _prog_cache[key]


def _run(inputs, trace=False):
    consts = _build_consts(inputs)
    word_ids = np.asarray(inputs["word_ids"])
    lengths = np.asarray(inputs["lengths"])

    preps = []
    for c in range(NCORES):
        sl = slice(c * PER, (c + 1) * PER)
        preps.append(_core_prep(word_ids[sl], lengths[sl]))
    Nt_max = np.stack([p[2] for p in preps]).max(0)
    W = tuple(int(min(NWORD, -(-int(n) // 16) * 16)) for n in Nt_max)

    lt_f8 = consts["LT"].astype(F8NP)
    ltr_f8 = consts["LTR"].astype(F8NP)
    in_maps = []
    for c in range(NCORES):
        wid_s, lens_s, _, perm = preps[c]
        in_maps.append({
            "oh": _build_onehot(wid_s, lens_s, W),
            "scidx": _build_scidx(perm),
            "lt": lt_f8, "ltr": ltr_f8,
            "w1": consts["W1eff"].astype(np.float16),
            "b1": consts["b1eff"],
            "w2": consts["W2eff"].astype(np.float16),
            "b2": consts["b2eff"],
            "wsc": consts["Wsc"].astype(np.float16),
            "bsc": np.full((1, 1), consts["bsc"], np.float32),
        })

    nc = _get_program(W)
    _spill_excess_waits(nc)  # idempotent; HW-compile only
    res = run_bass_kernel_spmd(nc, in_maps, list(range(NCORES)), trace=trace)
    out = np.concatenate([np.asarray(r["out"]).reshape(PER) for r in res.results])
    return out.reshape(B, 1).astype(np.float32), res.exec_time_ns


def kernel(**inputs):
    return _run(inputs)[0]
